# revision 1
# baseline (speedup 1.0000x reference)
"""Trainium2 Bass kernel for nn_DiscreteCommunication (GNN message passing).

v2 strategy (8 NeuronCores, SPMD single program, no collectives).

Key reduction: with hard=True straight-through Gumbel-softmax over 2 options,
the forward message is exactly one-hot, so only sign(z0 - z1 + g0 - g1)
matters. Define zd = featH @ (W_enc[evens] - W_enc[odds]).T  (64 cols) and
dg = ln(-ln(u0)+e) - ln(-ln(u1)+e) - (b_enc[evens]-b_enc[odds]) (host-
precomputed, bf16). Then m_even = (zd[src] >= dg); m_odd = 1 - m_even.

  - Host: sort edges by dst; device d owns dst nodes [2500d, 2500d+2500) as
    20 windows of 125 nodes. Edges padded to B 128-edge blocks per window
    (same B on all devices -> one SPMD program).
  - Phase Z: zd = featH_fp8 @ Wd_fp8.T over all 20096 padded nodes ->
    Zd table in local DRAM, bf16 rows padded to 128 cols (gather rows must
    be a multiple of 256B). Table is partition-major (row r = (n%128)*157 +
    n//128) so the table write is contiguous per partition; the gather
    index is host-transformed to match.
  - Phase MSG per window: dma_gather Zd[src'] (256B rows), m0 = (zg >= dg)
    bf16 (2x DVE), m1 = 1-m0, one-hot P = (iota == dstslot), segment-sum
    via PE: cps += m_b.T @ P_b, c = sign(cps) in bf16.
  - Phase GRU: node-parallel, all-bf16 matmuls (4x faster than f32 on PE),
    f32 elementwise/blend. dec folded: gi = feat@W_a.T + c@(W_ih[:,128:]
    @W_dec).T.
  - Host: concatenate the 8 per-device h_new slices.
"""
import os
import sys

sys.path.insert(0, "/opt/trn_rl_repo")

import numpy as np
import concourse.bacc as bacc
import concourse.mybir as mybir
import concourse.tile as tile
from concourse.bass_utils import run_bass_kernel_spmd

F32 = mybir.dt.float32
BF16 = mybir.dt.bfloat16
FP8 = mybir.dt.float8e4
I16 = mybir.dt.int16
AF = mybir.ActivationFunctionType
OP = mybir.AluOpType

N_NODES = 20000
HIDDEN = 128
MSG = 64
TWO_MSG = 2 * MSG  # 128
N_EDGES = 320000
EPS = 1e-10
NDEV = 8
WIN_NODES = 125            # nodes per window (<=128 for one-hot slots)
WINS = 20                  # windows per device
DEV_NODES = WIN_NODES * WINS   # 2500
N_WINDOWS = NDEV * WINS        # 160, covers all 20000 nodes exactly
ZBLKS = (N_NODES + 127) // 128  # 157 blocks over nodes (last partial: 32)
ZPAD = ZBLKS * 128              # 20096
GCH = 1024                      # gather call chunk (hard cap: 64 desc/engine/call)
SCRATCH = 49152                 # SWDGE ring: 3072 descriptors

_cache = {}


def build_program(B, phases="zmg", zero_bias=True, repeats=1,
                  skip_gather=False, skip_zmm=False, skip_zwrite=False,
                  skip_msgmm=False, strided_zwrite=True, zg_group=8):
    """Build the SPMD Bass program for B blocks-per-window."""
    nc = bacc.Bacc("TRN2", target_bir_lowering=False,
                   dynamic_dma_scratch_size=SCRATCH, num_swdge_queues=4)
    EW = B * 128               # padded edges per window
    EDEV = WINS * EW           # padded edges per device

    # ---- I/O ----
    # channel-major featH fp8: [p, zb, a, n] = featH_pad[zb*128+n, a*128+p]
    fh8 = nc.dram_tensor("fh8", [128, ZBLKS * 256], FP8, kind="ExternalInput")
    fh_locT = nc.dram_tensor("fh_locT", [128, WINS * 256], BF16, kind="ExternalInput")
    h_loc = nc.dram_tensor("h_loc", [DEV_NODES, HIDDEN], BF16, kind="ExternalInput")
    dg_g = nc.dram_tensor("dg_g", [128, WINS * B * MSG], FP8, kind="ExternalInput")
    src16 = nc.dram_tensor("src16", [128, EDEV // 16], I16, kind="ExternalInput")
    dstslot = nc.dram_tensor("dstslot", [128, WINS * B], BF16, kind="ExternalInput")
    wd8T = nc.dram_tensor("wd8T", [256, MSG], FP8, kind="ExternalInput")
    waT = nc.dram_tensor("waT", [128, 384], BF16, kind="ExternalInput")
    wbT = nc.dram_tensor("wbT", [128, 384], BF16, kind="ExternalInput")
    whhT = nc.dram_tensor("whhT", [128, 384], BF16, kind="ExternalInput")
    if not zero_bias:
        bias_rz = nc.dram_tensor("bias_rz", [128, 256], F32, kind="ExternalInput")
        bias_n = nc.dram_tensor("bias_n", [128, 128], F32, kind="ExternalInput")
        bias_hn = nc.dram_tensor("bias_hn", [128, 128], F32, kind="ExternalInput")
    h_new = nc.dram_tensor("h_new", [DEV_NODES, HIDDEN], F32, kind="ExternalOutput")

    with tile.TileContext(nc) as tc:
        with (
            tc.tile_pool(name="const", bufs=1) as cp,
        ):
            # ---- persistent constants ----
            iota_b = cp.tile([128, EW], BF16)
            # values 0..127 repeated B times along free dim; exact in bf16
            nc.gpsimd.iota(iota_b[:], pattern=[[0, B], [1, 128]], base=0,
                           channel_multiplier=0,
                           allow_small_or_imprecise_dtypes=True)
            dslot_t = cp.tile([128, WINS * B], BF16)
            nc.scalar.dma_start(out=dslot_t[:], in_=dstslot[:])
            wd_t = cp.tile([128, 2, MSG], FP8)
            nc.sync.dma_start(out=wd_t[:], in_=wd8T.rearrange("(a p) j -> p a j", p=128))
            waT_t = cp.tile([128, 384], BF16)
            nc.gpsimd.dma_start(out=waT_t[:], in_=waT[:])
            wbT_t = cp.tile([128, 384], BF16)
            nc.gpsimd.dma_start(out=wbT_t[:], in_=wbT[:])
            whhT_t = cp.tile([128, 384], BF16)
            nc.gpsimd.dma_start(out=whhT_t[:], in_=whhT[:])
            src16_t = cp.tile([128, EDEV // 16], I16)
            # biggest constant (0.7MB): keep it off the sync queue so it
            # overlaps the first fh8 feature loads in a single-eval launch
            nc.scalar.dma_start(out=src16_t[:], in_=src16[:])
            if not zero_bias:
                bias_rz_t = cp.tile([128, 256], F32)
                nc.sync.dma_start(out=bias_rz_t[:], in_=bias_rz[:])
                bias_n_t = cp.tile([128, 128], F32)
                nc.sync.dma_start(out=bias_n_t[:], in_=bias_n[:])
                bias_hn_t = cp.tile([128, 128], F32)
                nc.sync.dma_start(out=bias_hn_t[:], in_=bias_hn[:])
            cT_tiles = []
            for w in range(WINS):
                ct = cp.tile([128, 128], BF16, tag=f"cT{w}")
                cT_tiles.append(ct)

            # Zd table: row r=(n%128)*ZBLKS + n//128 (partition-major), 128
            # bf16 cols (0:64 = zd, 64:128 junk pad to reach the 256B-row
            # minimum of dma_gather).
            Zd = nc.dram_tensor("Zd", [ZPAD, TWO_MSG], BF16)
            Zdv = Zd.rearrange("(p g) j -> p (g j)", p=128)  # [128, ZBLKS*128]

            # ---- Phase Z: zd = featH_fp8 @ Wd_fp8.T (all nodes) ----
            ZG = zg_group
            def emit_z_phase():
             with (
                tc.tile_pool(name="zio", bufs=3) as zio,
                tc.tile_pool(name="zps", bufs=2, space="PSUM") as zps,
             ):
              engs = [nc.sync, nc.scalar, nc.gpsimd]
              for gi_, g0 in enumerate(range(0, ZBLKS, ZG)):
                gn = min(ZG, ZBLKS - g0)
                fg = zio.tile([128, ZG, 2, 128], FP8, tag="fg")
                cols = gn * 256
                engs[gi_ % 3].dma_start(
                    out=fg[:].rearrange("p g a n -> p (g a n)")[:, :cols],
                    in_=fh8[:, g0 * 256 : g0 * 256 + cols])
                zp = zps.tile([128, ZG * MSG], F32, space="PSUM", tag="zp")
                if not skip_zmm:
                    for zi in range(gn):
                        zslc = zp[:, zi * MSG : (zi + 1) * MSG]
                        nc.tensor.matmul(out=zslc, lhsT=fg[:, zi, 0, :],
                                         rhs=wd_t[:, 0, :], start=True, stop=False)
                        nc.tensor.matmul(out=zslc, lhsT=fg[:, zi, 1, :],
                                         rhs=wd_t[:, 1, :], start=False, stop=True)
                else:
                    nc.vector.memset(zp[:, : gn * MSG], 0.0)
                zs = zio.tile([128, ZG, TWO_MSG], BF16, tag="zs")
                if not strided_zwrite:
                    # define the pad cols so the table-write DMA reads
                    # initialized memory (cols 64:128 are never consumed)
                    nc.vector.memset(zs[:, :, MSG:TWO_MSG], 0.0)
                nc.scalar.copy(
                    out=zs[:, :gn, 0:MSG],
                    in_=zp[:, : gn * MSG].rearrange("p (g j) -> p g j", g=gn))
                if not skip_zwrite:
                    weng = engs[(gi_ + 1) % 3]
                    if strided_zwrite:
                        weng.dma_start(
                            out=Zd.rearrange("(p g) j -> p g j", p=128)[:, g0 : g0 + gn, 0:MSG],
                            in_=zs[:, :gn, 0:MSG])
                    else:
                        weng.dma_start(
                            out=Zdv[:, g0 * TWO_MSG : (g0 + gn) * TWO_MSG],
                            in_=zs[:, :gn, :].rearrange("p g j -> p (g j)"))

            # ---- Phase MSG + GRU, interleaved ----
            WG = 4
            qctr = [0]
            def emit_msg_window(w):
                zg = gp.tile([128, B, TWO_MSG], BF16, tag="zg")
                if skip_gather:
                    nc.vector.memset(zg[:], 0.0)
                else:
                    off = 0
                    while off < EW:
                        chunk = min(GCH, EW - off)
                        nc.gpsimd.dma_gather(
                            zg[:, off // 128 : (off + chunk) // 128, :], Zd[:],
                            src16_t[:, (w * EW + off) // 16 : (w * EW + off + chunk) // 16],
                            num_idxs=chunk, num_idxs_reg=chunk, elem_size=TWO_MSG,
                            queue_num=qctr[0] % 4,
                        )
                        qctr[0] += 1
                        off += chunk
                dgw = gp.tile([128, B, MSG], FP8, tag="dgw")
                ueng = nc.sync if w % 2 == 0 else nc.scalar
                ueng.dma_start(
                    out=dgw[:].rearrange("p b c -> p (b c)"),
                    in_=dg_g[:, w * B * MSG : (w + 1) * B * MSG])
                m = mp.tile([128, B, TWO_MSG], BF16, tag="m")
                nc.vector.tensor_tensor(out=m[:, :, 0:MSG], in0=zg[:, :, 0:MSG],
                                        in1=dgw[:], op=OP.is_ge)
                # m1 = 1 - m0 == (m0 < 1)
                nc.vector.tensor_scalar(out=m[:, :, MSG:TWO_MSG], in0=m[:, :, 0:MSG],
                                        scalar1=1.0, scalar2=None, op0=OP.is_lt)
                P = pp.tile([128, B, 128], BF16, tag="P")
                nc.vector.tensor_tensor(
                    out=P[:],
                    in0=iota_b[:].rearrange("p (b j) -> p b j", b=B),
                    in1=dslot_t[:, w * B : (w + 1) * B, None].to_broadcast([128, B, 128]),
                    op=OP.is_equal)
                cps = mps.tile([128, 128], F32, space="PSUM", tag="cps")
                if not skip_msgmm:
                    for b in range(B):
                        nc.tensor.matmul(out=cps[:], lhsT=m[:, b, :], rhs=P[:, b, :],
                                         start=(b == 0), stop=(b == B - 1))
                else:
                    nc.vector.memset(cps[:], 0.0)
                # c = (c_sum > 0) == Sign(c_sum) since c_sum >= 0; runs on ACT
                nc.scalar.sign(out=cT_tiles[w][:], in_=cps[:])

            def emit_gru_group(w0):
                xh = rp.tile([128, WG, 2, 128], BF16, tag="xh")
                nc.sync.dma_start(
                    out=xh[:].rearrange("p w a n -> p (w a n)"),
                    in_=fh_locT[:, w0 * 256 : (w0 + WG) * 256])
                hl = rp.tile([128, WG, 128], BF16, tag="hl")
                for wi in range(WG):
                    w = w0 + wi
                    nc.scalar.dma_start(
                        out=hl[:WIN_NODES, wi, :],
                        in_=h_loc[w * WIN_NODES : (w + 1) * WIN_NODES, :])
                # two PSUM groups per window (rz gates / n gate) so each
                # group is opened and closed over exactly the same col range
                gi = rps.tile([128, WG, 256], F32, space="PSUM", tag="gi")
                gn_ps = rps.tile([128, WG, 128], F32, space="PSUM", tag="gn_ps")
                hn_ps = rps2.tile([128, WG, 128], F32, space="PSUM", tag="hn_ps")
                for wi in range(WG):
                    w = w0 + wi
                    nc.tensor.matmul(out=gi[:, wi, :], lhsT=xh[:, wi, 0, :],
                                     rhs=waT_t[:, 0:256], start=True, stop=False)
                    nc.tensor.matmul(out=gi[:, wi, :], lhsT=cT_tiles[w][:],
                                     rhs=wbT_t[:, 0:256], start=False, stop=False)
                    nc.tensor.matmul(out=gi[:, wi, :], lhsT=xh[:, wi, 1, :],
                                     rhs=whhT_t[:, 0:256], start=False, stop=True)
                    nc.tensor.matmul(out=gn_ps[:, wi, :], lhsT=xh[:, wi, 0, :],
                                     rhs=waT_t[:, 256:384], start=True, stop=False)
                    nc.tensor.matmul(out=gn_ps[:, wi, :], lhsT=cT_tiles[w][:],
                                     rhs=wbT_t[:, 256:384], start=False, stop=True)
                    nc.tensor.matmul(out=hn_ps[:, wi, :], lhsT=xh[:, wi, 1, :],
                                     rhs=whhT_t[:, 256:384], start=True, stop=True)
                V = WIN_NODES
                rz_s = rp.tile([128, WG, 256], F32, tag="rz_s")
                if zero_bias:
                    nc.scalar.activation(rz_s[:V], gi[:V], AF.Sigmoid)
                    rhn = rp.tile([128, WG, 128], F32, tag="rhn")
                    nc.vector.tensor_tensor(out=rhn[:V], in0=rz_s[:V, :, 0:128],
                                            in1=hn_ps[:V], op=OP.mult)
                    narg = rp.tile([128, WG, 128], F32, tag="narg")
                    nc.vector.tensor_tensor(out=narg[:V], in0=rhn[:V],
                                            in1=gn_ps[:V], op=OP.add)
                else:
                    rz = rp.tile([128, WG, 256], F32, tag="rz")
                    nc.vector.tensor_tensor(
                        out=rz[:V], in0=gi[:V],
                        in1=bias_rz_t[:V, None, :].to_broadcast([V, WG, 256]), op=OP.add)
                    nc.scalar.activation(rz_s[:V], rz[:V], AF.Sigmoid)
                    hn = rp.tile([128, WG, 128], F32, tag="hn")
                    nc.vector.tensor_tensor(
                        out=hn[:V], in0=hn_ps[:V],
                        in1=bias_hn_t[:V, None, :].to_broadcast([V, WG, 128]), op=OP.add)
                    inn = rp.tile([128, WG, 128], F32, tag="inn")
                    nc.vector.tensor_tensor(
                        out=inn[:V], in0=gn_ps[:V],
                        in1=bias_n_t[:V, None, :].to_broadcast([V, WG, 128]), op=OP.add)
                    rhn = rp.tile([128, WG, 128], F32, tag="rhn")
                    nc.vector.tensor_tensor(out=rhn[:V], in0=rz_s[:V, :, 0:128], in1=hn[:V], op=OP.mult)
                    narg = rp.tile([128, WG, 128], F32, tag="narg")
                    nc.vector.tensor_tensor(out=narg[:V], in0=inn[:V], in1=rhn[:V], op=OP.add)
                n_t = rp.tile([128, WG, 128], F32, tag="n_t")
                nc.scalar.activation(n_t[:V], narg[:V], AF.Tanh)
                hmn = rp.tile([128, WG, 128], F32, tag="hmn")
                nc.vector.tensor_tensor(out=hmn[:V], in0=hl[:V], in1=n_t[:V], op=OP.subtract)
                zh = rp.tile([128, WG, 128], F32, tag="zh")
                nc.vector.tensor_tensor(out=zh[:V], in0=rz_s[:V, :, 128:256], in1=hmn[:V], op=OP.mult)
                ho = rp.tile([128, WG, 128], F32, tag="ho")
                nc.vector.tensor_tensor(out=ho[:V], in0=n_t[:V], in1=zh[:V], op=OP.add)
                for wi in range(WG):
                    w = w0 + wi
                    nc.sync.dma_start(
                        out=h_new[w * WIN_NODES : (w + 1) * WIN_NODES, :],
                        in_=ho[:WIN_NODES, wi, :])

            for _rep in range(repeats):
                if "z" in phases:
                    emit_z_phase()
                with (
                    tc.tile_pool(name="msg", bufs=2) as mp,
                    tc.tile_pool(name="ponehot", bufs=3) as pp,
                    tc.tile_pool(name="gat", bufs=3) as gp,
                    tc.tile_pool(name="mps", bufs=2, space="PSUM") as mps,
                    tc.tile_pool(name="gru", bufs=2) as rp,
                    tc.tile_pool(name="rps", bufs=1, space="PSUM") as rps,
                    tc.tile_pool(name="rps2", bufs=2, space="PSUM") as rps2,
                ):
                    for w in range(WINS):
                        if "m" in phases:
                            emit_msg_window(w)
                        if "g" in phases and w % WG == WG - 1:
                            emit_gru_group(w - WG + 1)

    nc.compile()
    return nc


# message-column permutation: evens first, then odds
PERM = np.concatenate([np.arange(0, TWO_MSG, 2), np.arange(1, TWO_MSG, 2)])


def _prep_host(feat, h, src, dst, u, dg_bias=None):
    """Host-side sharding/layout. Returns (B, list of per-core in_maps)."""
    import ml_dtypes
    bf16 = ml_dtypes.bfloat16
    fp8 = ml_dtypes.float8_e4m3

    feat = np.ascontiguousarray(feat, dtype=np.float32)
    h = np.ascontiguousarray(h, dtype=np.float32)
    src = np.asarray(src).astype(np.int64)
    dst = np.asarray(dst).astype(np.int64)
    u = np.asarray(u, dtype=np.float32)

    featH = np.concatenate([feat, h], axis=1)  # [N, 256]
    featH_pad = np.zeros((ZPAD, 256), np.float32)
    featH_pad[:N_NODES] = featH
    # channel-major fp8: [p, zb, a, n] = featH_pad[zb*128+n, a*128+p]
    fh8 = np.ascontiguousarray(
        featH_pad.astype(fp8).reshape(ZBLKS, 128, 2, 128).transpose(3, 0, 2, 1)
    ).reshape(128, -1)

    # host Gumbel difference, folding b_enc (b_enc=0 here but stay general at
    # the call site via _prep_weights -> dg_bias)
    dgf = (np.log(-np.log(u[..., 0].astype(np.float64) + EPS) + EPS)
           - np.log(-np.log(u[..., 1].astype(np.float64) + EPS) + EPS))
    if dg_bias is not None:
        dgf = dgf - np.asarray(dg_bias, np.float64)[None, :]

    order = np.argsort(dst, kind="stable")
    dst_s = dst[order]
    src_s = src[order]
    win = dst_s // WIN_NODES                     # window id per sorted edge
    counts = np.bincount(win, minlength=N_WINDOWS)
    starts = np.zeros(N_WINDOWS + 1, np.int64)
    np.cumsum(counts, out=starts[1:])
    B = int(np.max((counts + 127) // 128))
    B = max(B, 1)
    EW = B * 128
    EDEV = WINS * EW

    in_maps = []
    for d in range(NDEV):
        src_pad = np.zeros((EDEV,), np.int64)
        slot_pad = np.full((EDEV,), -1.0, np.float32)
        dg_pad = np.zeros((EDEV, MSG), np.float32)
        for k in range(WINS):
            wid = d * WINS + k
            s, e = starts[wid], starts[wid + 1]
            n = e - s
            o = k * EW
            src_pad[o : o + n] = src_s[s:e]
            slot_pad[o : o + n] = (dst_s[s:e] - wid * WIN_NODES).astype(np.float32)
            dg_pad[o : o + n] = dgf[order[s:e]]

        # partition-major table index: row of node n is (n%128)*ZBLKS + n//128
        srcT = (src_pad % 128) * ZBLKS + src_pad // 128
        # gather idx layout: [p, s] = idx[16*s + p%16], replicated across groups
        idx16 = np.empty((128, EDEV // 16), np.int16)
        flat = srcT.astype(np.int16).reshape(EDEV // 16, 16).T  # [16, EDEV/16]
        for g in range(8):
            idx16[g * 16 : (g + 1) * 16, :] = flat
        # compact dstslot: [p, w*B + b] = slot of edge (w, b, p)
        dstslot_c = np.ascontiguousarray(
            slot_pad.reshape(WINS * B, 128).T.astype(bf16))
        # dg swizzled: [p, blk*64 + c] = dg_pad[blk*128 + p, c]
        dg_sw = np.ascontiguousarray(
            dg_pad.astype(fp8).reshape(EDEV // 128, 128, MSG)
            .transpose(1, 0, 2)).reshape(128, -1)
        # local featH channel-major bf16: [p, w, a, n] = featH[node, a*128+p]
        base = d * DEV_NODES
        end = min(base + DEV_NODES, N_NODES)
        locflat = np.zeros((DEV_NODES, 256), np.float32)
        locflat[: end - base] = featH[base:end]
        loc = np.zeros((WINS, 128, 2, 128), np.float32)  # [w, n, a, p]
        loc[:, :WIN_NODES] = locflat.reshape(WINS, WIN_NODES, 2, 128)
        fh_locT = np.ascontiguousarray(
            loc.astype(bf16).transpose(3, 0, 2, 1)).reshape(128, -1)
        h_pad = np.zeros((DEV_NODES, HIDDEN), np.float32)
        h_pad[: end - base] = h[base:end]
        h_loc = np.ascontiguousarray(h_pad.astype(bf16))
        in_maps.append({
            "fh8": fh8, "fh_locT": fh_locT, "h_loc": h_loc,
            "dg_g": dg_sw, "src16": idx16, "dstslot": dstslot_c,
        })
    return B, in_maps


def _prep_weights(W_enc, b_enc, W_dec, b_dec, W_ih, W_hh, b_ih, b_hh):
    import ml_dtypes
    bf16 = ml_dtypes.bfloat16
    fp8 = ml_dtypes.float8_e4m3

    W_enc = np.asarray(W_enc, np.float32)
    W_dec = np.asarray(W_dec, np.float32)
    W_ih = np.asarray(W_ih, np.float32)
    W_hh = np.asarray(W_hh, np.float32)
    b_enc = np.asarray(b_enc, np.float32)
    b_dec = np.asarray(b_dec, np.float32)
    b_ih = np.asarray(b_ih, np.float32)
    b_hh = np.asarray(b_hh, np.float32)

    Wd = W_enc[0::2].astype(np.float64) - W_enc[1::2].astype(np.float64)  # [64, 256]
    wd8T = np.ascontiguousarray(Wd.T.astype(fp8))            # [256, 64]

    W_b = (W_ih[:, HIDDEN:].astype(np.float64) @ W_dec.astype(np.float64))
    b_comb = (W_ih[:, HIDDEN:].astype(np.float64) @ b_dec.astype(np.float64)) + b_ih

    waT = np.ascontiguousarray(W_ih[:, :HIDDEN].T.astype(bf16))      # [128, 384]
    wbT = np.ascontiguousarray(W_b.T.astype(np.float32)[PERM, :].astype(bf16))
    whhT = np.ascontiguousarray(W_hh.T.astype(bf16))                 # [128, 384]
    brz = (b_comb[:256] + b_hh[:256]).astype(np.float32)
    bn = b_comb[256:384].astype(np.float32)
    bhn = b_hh[256:384].astype(np.float32)
    return {
        "wd8T": wd8T,
        "waT": waT, "wbT": wbT, "whhT": whhT,
        "bias_rz": np.ascontiguousarray(np.tile(brz, (128, 1))),
        "bias_n": np.ascontiguousarray(np.tile(bn, (128, 1))),
        "bias_hn": np.ascontiguousarray(np.tile(bhn, (128, 1))),
    }, (b_enc[0::2].astype(np.float64) - b_enc[1::2].astype(np.float64))


def kernel(feat, h, src, dst, u, W_enc, b_enc, W_dec, b_dec, W_ih, W_hh,
           b_ih, b_hh):
    wmap, dg_bias = _prep_weights(W_enc, b_enc, W_dec, b_dec, W_ih, W_hh,
                                  b_ih, b_hh)
    B, in_maps = _prep_host(feat, h, src, dst, u, dg_bias=dg_bias)
    for m in in_maps:
        m.update(wmap)

    phases = os.environ.get("KERNEL_PHASES", "zmg")
    zero_bias = not (np.any(np.asarray(b_dec)) or np.any(np.asarray(b_ih))
                     or np.any(np.asarray(b_hh)))
    key = (B, phases, zero_bias)
    if key not in _cache:
        _cache[key] = build_program(B, phases, zero_bias)
    nc = _cache[key]

    res = run_bass_kernel_spmd(nc, in_maps, core_ids=list(range(NDEV)))
    h_new = np.concatenate([res.results[d]["h_new"] for d in range(NDEV)],
                           axis=0)[:N_NODES]
    return (h_new, h_new)



# revision 5
# speedup vs baseline: 4.3561x; 4.3561x over previous
"""Trainium2 Bass kernel for nn_DiscreteCommunication (GNN message passing).

v2 strategy (8 NeuronCores, SPMD single program, no collectives).

Key reduction: with hard=True straight-through Gumbel-softmax over 2 options,
the forward message is exactly one-hot, so only sign(z0 - z1 + g0 - g1)
matters. Define zd = featH @ (W_enc[evens] - W_enc[odds]).T  (64 cols) and
dg = ln(-ln(u0)+e) - ln(-ln(u1)+e) - (b_enc[evens]-b_enc[odds]) (host-
precomputed, bf16). Then m_even = (zd[src] >= dg); m_odd = 1 - m_even.

  - Host: sort edges by dst; device d owns dst nodes [2500d, 2500d+2500) as
    20 windows of 125 nodes. Edges padded to B 128-edge blocks per window
    (same B on all devices -> one SPMD program).
  - Phase Z: zd = featH_fp8 @ Wd_fp8.T over all 20096 padded nodes ->
    Zd table in local DRAM, bf16 rows padded to 128 cols (gather rows must
    be a multiple of 256B). Table is partition-major (row r = (n%128)*157 +
    n//128) so the table write is contiguous per partition; the gather
    index is host-transformed to match.
  - Phase MSG per window: dma_gather Zd[src'] (256B rows), m0 = (zg >= dg)
    bf16 (2x DVE), m1 = 1-m0, one-hot P = (iota == dstslot), segment-sum
    via PE: cps += m_b.T @ P_b, c = sign(cps) in bf16.
  - Phase GRU: node-parallel, all-bf16 matmuls (4x faster than f32 on PE),
    f32 elementwise/blend. dec folded: gi = feat@W_a.T + c@(W_ih[:,128:]
    @W_dec).T.
  - Host: concatenate the 8 per-device h_new slices.
"""
import os
import sys

sys.path.insert(0, "/opt/trn_rl_repo")

import numpy as np
import concourse.bacc as bacc
import concourse.mybir as mybir
import concourse.tile as tile
from concourse.bass_utils import run_bass_kernel_spmd

F32 = mybir.dt.float32
BF16 = mybir.dt.bfloat16
FP8 = mybir.dt.float8e4
I16 = mybir.dt.int16
AF = mybir.ActivationFunctionType
OP = mybir.AluOpType

N_NODES = 20000
HIDDEN = 128
MSG = 64
TWO_MSG = 2 * MSG  # 128
N_EDGES = 320000
EPS = 1e-10
NDEV = 8
WIN_NODES = 125            # nodes per window (<=128 for one-hot slots)
WINS = 20                  # windows per device
DEV_NODES = WIN_NODES * WINS   # 2500
N_WINDOWS = NDEV * WINS        # 160, covers all 20000 nodes exactly
ZBLKS = (N_NODES + 127) // 128  # 157 blocks over nodes (last partial: 32)
ZPAD = ZBLKS * 128              # 20096
GCH = 1024                      # gather call chunk (hard cap: 64 desc/engine/call)
SCRATCH = 49152                 # SWDGE ring: 3072 descriptors

_cache = {}


def build_program(B, phases="zmg", zero_bias=True, repeats=1,
                  skip_gather=False, skip_zmm=False, skip_zwrite=False,
                  skip_msgmm=False, strided_zwrite=True, zg_group=8,
                  hw_loop=True):
    """Build the SPMD Bass program for B blocks-per-window."""
    nc = bacc.Bacc("TRN2", target_bir_lowering=False,
                   dynamic_dma_scratch_size=SCRATCH, num_swdge_queues=4)
    EW = B * 128               # padded edges per window
    EDEV = WINS * EW           # padded edges per device

    # ---- I/O ----
    # channel-major featH fp8: [p, zb, a, n] = featH_pad[zb*128+n, a*128+p]
    fh8 = nc.dram_tensor("fh8", [128, ZBLKS * 256], FP8, kind="ExternalInput")
    fh_locT = nc.dram_tensor("fh_locT", [128, WINS * 256], BF16, kind="ExternalInput")
    h_loc = nc.dram_tensor("h_loc", [DEV_NODES, HIDDEN], BF16, kind="ExternalInput")
    dg_g = nc.dram_tensor("dg_g", [128, WINS * B * MSG], FP8, kind="ExternalInput")
    src16 = nc.dram_tensor("src16", [128, EDEV // 16], I16, kind="ExternalInput")
    dstslot = nc.dram_tensor("dstslot", [128, WINS * B], BF16, kind="ExternalInput")
    wd8T = nc.dram_tensor("wd8T", [256, MSG], FP8, kind="ExternalInput")
    waT = nc.dram_tensor("waT", [128, 384], BF16, kind="ExternalInput")
    wbT = nc.dram_tensor("wbT", [128, 384], BF16, kind="ExternalInput")
    whhT = nc.dram_tensor("whhT", [128, 384], BF16, kind="ExternalInput")
    if not zero_bias:
        bias_rz = nc.dram_tensor("bias_rz", [128, 256], F32, kind="ExternalInput")
        bias_n = nc.dram_tensor("bias_n", [128, 128], F32, kind="ExternalInput")
        bias_hn = nc.dram_tensor("bias_hn", [128, 128], F32, kind="ExternalInput")
    h_new = nc.dram_tensor("h_new", [DEV_NODES, HIDDEN], F32, kind="ExternalOutput")

    with tile.TileContext(nc) as tc:
        with (
            tc.tile_pool(name="const", bufs=1) as cp,
        ):
            # ---- persistent constants ----
            iota_b = cp.tile([128, EW], BF16)
            # values 0..127 repeated B times along free dim; exact in bf16
            nc.gpsimd.iota(iota_b[:], pattern=[[0, B], [1, 128]], base=0,
                           channel_multiplier=0,
                           allow_small_or_imprecise_dtypes=True)
            dslot_t = cp.tile([128, WINS * B], BF16)
            nc.scalar.dma_start(out=dslot_t[:], in_=dstslot[:])
            wd_t = cp.tile([128, 2, MSG], FP8)
            nc.sync.dma_start(out=wd_t[:], in_=wd8T.rearrange("(a p) j -> p a j", p=128))
            waT_t = cp.tile([128, 384], BF16)
            nc.gpsimd.dma_start(out=waT_t[:], in_=waT[:])
            wbT_t = cp.tile([128, 384], BF16)
            nc.gpsimd.dma_start(out=wbT_t[:], in_=wbT[:])
            whhT_t = cp.tile([128, 384], BF16)
            nc.gpsimd.dma_start(out=whhT_t[:], in_=whhT[:])
            src16_t = cp.tile([128, EDEV // 16], I16)
            # biggest constant (0.7MB): keep it off the sync queue so it
            # overlaps the first fh8 feature loads in a single-eval launch
            nc.scalar.dma_start(out=src16_t[:], in_=src16[:])
            if not zero_bias:
                bias_rz_t = cp.tile([128, 256], F32)
                nc.sync.dma_start(out=bias_rz_t[:], in_=bias_rz[:])
                bias_n_t = cp.tile([128, 128], F32)
                nc.sync.dma_start(out=bias_n_t[:], in_=bias_n[:])
                bias_hn_t = cp.tile([128, 128], F32)
                nc.sync.dma_start(out=bias_hn_t[:], in_=bias_hn[:])
            cT_tiles = []
            for w in range(WINS):
                ct = cp.tile([128, 128], BF16, tag=f"cT{w}")
                cT_tiles.append(ct)

            # Zd table: row r=(n%128)*ZBLKS + n//128 (partition-major), 128
            # bf16 cols (0:64 = zd, 64:128 junk pad to reach the 256B-row
            # minimum of dma_gather).
            Zd = nc.dram_tensor("Zd", [ZPAD, TWO_MSG], BF16)
            Zdv = Zd.rearrange("(p g) j -> p (g j)", p=128)  # [128, ZBLKS*128]

            # ---- Phase Z: zd = featH_fp8 @ Wd_fp8.T (all nodes) ----
            ZG = zg_group
            def emit_z_phase():
             with (
                tc.tile_pool(name="zio", bufs=3) as zio,
                tc.tile_pool(name="zps", bufs=2, space="PSUM") as zps,
             ):
              engs = [nc.sync, nc.scalar, nc.gpsimd]
              for gi_, g0 in enumerate(range(0, ZBLKS, ZG)):
                gn = min(ZG, ZBLKS - g0)
                fg = zio.tile([128, ZG, 2, 128], FP8, tag="fg")
                cols = gn * 256
                engs[gi_ % 3].dma_start(
                    out=fg[:].rearrange("p g a n -> p (g a n)")[:, :cols],
                    in_=fh8[:, g0 * 256 : g0 * 256 + cols])
                zp = zps.tile([128, ZG * MSG], F32, space="PSUM", tag="zp")
                if not skip_zmm:
                    for zi in range(gn):
                        zslc = zp[:, zi * MSG : (zi + 1) * MSG]
                        nc.tensor.matmul(out=zslc, lhsT=fg[:, zi, 0, :],
                                         rhs=wd_t[:, 0, :], start=True, stop=False)
                        nc.tensor.matmul(out=zslc, lhsT=fg[:, zi, 1, :],
                                         rhs=wd_t[:, 1, :], start=False, stop=True)
                else:
                    nc.vector.memset(zp[:, : gn * MSG], 0.0)
                zs = zio.tile([128, ZG, TWO_MSG], BF16, tag="zs")
                if not strided_zwrite:
                    # define the pad cols so the table-write DMA reads
                    # initialized memory (cols 64:128 are never consumed)
                    nc.vector.memset(zs[:, :, MSG:TWO_MSG], 0.0)
                nc.scalar.copy(
                    out=zs[:, :gn, 0:MSG],
                    in_=zp[:, : gn * MSG].rearrange("p (g j) -> p g j", g=gn))
                if not skip_zwrite:
                    weng = engs[(gi_ + 1) % 3]
                    if strided_zwrite:
                        weng.dma_start(
                            out=Zd.rearrange("(p g) j -> p g j", p=128)[:, g0 : g0 + gn, 0:MSG],
                            in_=zs[:, :gn, 0:MSG])
                    else:
                        weng.dma_start(
                            out=Zdv[:, g0 * TWO_MSG : (g0 + gn) * TWO_MSG],
                            in_=zs[:, :gn, :].rearrange("p g j -> p (g j)"))

            # ---- Phase MSG + GRU, interleaved ----
            WG = 4
            qctr = [0]
            mp = pp = gp = mps = rp = rps = rps2 = None
            def emit_msg_window(w):
                zg = gp.tile([128, B, TWO_MSG], BF16, tag="zg")
                if skip_gather:
                    nc.vector.memset(zg[:], 0.0)
                else:
                    off = 0
                    while off < EW:
                        chunk = min(GCH, EW - off)
                        nc.gpsimd.dma_gather(
                            zg[:, off // 128 : (off + chunk) // 128, :], Zd[:],
                            src16_t[:, (w * EW + off) // 16 : (w * EW + off + chunk) // 16],
                            num_idxs=chunk, num_idxs_reg=chunk, elem_size=TWO_MSG,
                            queue_num=qctr[0] % 4,
                        )
                        qctr[0] += 1
                        off += chunk
                dgw = gp.tile([128, B, MSG], FP8, tag="dgw")
                ueng = nc.sync if w % 2 == 0 else nc.scalar
                ueng.dma_start(
                    out=dgw[:].rearrange("p b c -> p (b c)"),
                    in_=dg_g[:, w * B * MSG : (w + 1) * B * MSG])
                m = mp.tile([128, B, TWO_MSG], BF16, tag="m")
                nc.vector.tensor_tensor(out=m[:, :, 0:MSG], in0=zg[:, :, 0:MSG],
                                        in1=dgw[:], op=OP.is_ge)
                # m1 = 1 - m0 == (m0 < 1)
                nc.vector.tensor_scalar(out=m[:, :, MSG:TWO_MSG], in0=m[:, :, 0:MSG],
                                        scalar1=1.0, scalar2=None, op0=OP.is_lt)
                P = pp.tile([128, B, 128], BF16, tag="P")
                nc.vector.tensor_tensor(
                    out=P[:],
                    in0=iota_b[:].rearrange("p (b j) -> p b j", b=B),
                    in1=dslot_t[:, w * B : (w + 1) * B, None].to_broadcast([128, B, 128]),
                    op=OP.is_equal)
                cps = mps.tile([128, 128], F32, space="PSUM", tag="cps")
                if not skip_msgmm:
                    for b in range(B):
                        nc.tensor.matmul(out=cps[:], lhsT=m[:, b, :], rhs=P[:, b, :],
                                         start=(b == 0), stop=(b == B - 1))
                else:
                    nc.vector.memset(cps[:], 0.0)
                # c = (c_sum > 0) == Sign(c_sum) since c_sum >= 0; runs on ACT
                nc.scalar.sign(out=cT_tiles[w][:], in_=cps[:])

            def emit_gru_group(w0):
                xh = rp.tile([128, WG, 2, 128], BF16, tag="xh")
                nc.sync.dma_start(
                    out=xh[:].rearrange("p w a n -> p (w a n)"),
                    in_=fh_locT[:, w0 * 256 : (w0 + WG) * 256])
                hl = rp.tile([128, WG, 128], BF16, tag="hl")
                for wi in range(WG):
                    w = w0 + wi
                    nc.scalar.dma_start(
                        out=hl[:WIN_NODES, wi, :],
                        in_=h_loc[w * WIN_NODES : (w + 1) * WIN_NODES, :])
                # two PSUM groups per window (rz gates / n gate) so each
                # group is opened and closed over exactly the same col range
                gi = rps.tile([128, WG, 256], F32, space="PSUM", tag="gi")
                gn_ps = rps.tile([128, WG, 128], F32, space="PSUM", tag="gn_ps")
                hn_ps = rps2.tile([128, WG, 128], F32, space="PSUM", tag="hn_ps")
                for wi in range(WG):
                    w = w0 + wi
                    nc.tensor.matmul(out=gi[:, wi, :], lhsT=xh[:, wi, 0, :],
                                     rhs=waT_t[:, 0:256], start=True, stop=False)
                    nc.tensor.matmul(out=gi[:, wi, :], lhsT=cT_tiles[w][:],
                                     rhs=wbT_t[:, 0:256], start=False, stop=False)
                    nc.tensor.matmul(out=gi[:, wi, :], lhsT=xh[:, wi, 1, :],
                                     rhs=whhT_t[:, 0:256], start=False, stop=True)
                    nc.tensor.matmul(out=gn_ps[:, wi, :], lhsT=xh[:, wi, 0, :],
                                     rhs=waT_t[:, 256:384], start=True, stop=False)
                    nc.tensor.matmul(out=gn_ps[:, wi, :], lhsT=cT_tiles[w][:],
                                     rhs=wbT_t[:, 256:384], start=False, stop=True)
                    nc.tensor.matmul(out=hn_ps[:, wi, :], lhsT=xh[:, wi, 1, :],
                                     rhs=whhT_t[:, 256:384], start=True, stop=True)
                V = WIN_NODES
                rz_s = rp.tile([128, WG, 256], F32, tag="rz_s")
                if zero_bias:
                    nc.scalar.activation(rz_s[:V], gi[:V], AF.Sigmoid)
                    rhn = rp.tile([128, WG, 128], F32, tag="rhn")
                    nc.vector.tensor_tensor(out=rhn[:V], in0=rz_s[:V, :, 0:128],
                                            in1=hn_ps[:V], op=OP.mult)
                    narg = rp.tile([128, WG, 128], F32, tag="narg")
                    nc.vector.tensor_tensor(out=narg[:V], in0=rhn[:V],
                                            in1=gn_ps[:V], op=OP.add)
                else:
                    rz = rp.tile([128, WG, 256], F32, tag="rz")
                    nc.vector.tensor_tensor(
                        out=rz[:V], in0=gi[:V],
                        in1=bias_rz_t[:V, None, :].to_broadcast([V, WG, 256]), op=OP.add)
                    nc.scalar.activation(rz_s[:V], rz[:V], AF.Sigmoid)
                    hn = rp.tile([128, WG, 128], F32, tag="hn")
                    nc.vector.tensor_tensor(
                        out=hn[:V], in0=hn_ps[:V],
                        in1=bias_hn_t[:V, None, :].to_broadcast([V, WG, 128]), op=OP.add)
                    inn = rp.tile([128, WG, 128], F32, tag="inn")
                    nc.vector.tensor_tensor(
                        out=inn[:V], in0=gn_ps[:V],
                        in1=bias_n_t[:V, None, :].to_broadcast([V, WG, 128]), op=OP.add)
                    rhn = rp.tile([128, WG, 128], F32, tag="rhn")
                    nc.vector.tensor_tensor(out=rhn[:V], in0=rz_s[:V, :, 0:128], in1=hn[:V], op=OP.mult)
                    narg = rp.tile([128, WG, 128], F32, tag="narg")
                    nc.vector.tensor_tensor(out=narg[:V], in0=inn[:V], in1=rhn[:V], op=OP.add)
                n_t = rp.tile([128, WG, 128], F32, tag="n_t")
                nc.scalar.activation(n_t[:V], narg[:V], AF.Tanh)
                hmn = rp.tile([128, WG, 128], F32, tag="hmn")
                nc.vector.tensor_tensor(out=hmn[:V], in0=hl[:V], in1=n_t[:V], op=OP.subtract)
                zh = rp.tile([128, WG, 128], F32, tag="zh")
                nc.vector.tensor_tensor(out=zh[:V], in0=rz_s[:V, :, 128:256], in1=hmn[:V], op=OP.mult)
                ho = rp.tile([128, WG, 128], F32, tag="ho")
                nc.vector.tensor_tensor(out=ho[:V], in0=n_t[:V], in1=zh[:V], op=OP.add)
                for wi in range(WG):
                    w = w0 + wi
                    nc.sync.dma_start(
                        out=h_new[w * WIN_NODES : (w + 1) * WIN_NODES, :],
                        in_=ho[:WIN_NODES, wi, :])

            def emit_eval():
                nonlocal mp, pp, gp, mps, rp, rps, rps2
                if "z" in phases:
                    emit_z_phase()
                with (
                    tc.tile_pool(name="msg", bufs=2) as mp,
                    tc.tile_pool(name="ponehot", bufs=3) as pp,
                    tc.tile_pool(name="gat", bufs=3) as gp,
                    tc.tile_pool(name="mps", bufs=2, space="PSUM") as mps,
                    tc.tile_pool(name="gru", bufs=2) as rp,
                    tc.tile_pool(name="rps", bufs=1, space="PSUM") as rps,
                    tc.tile_pool(name="rps2", bufs=2, space="PSUM") as rps2,
                ):
                    for w in range(WINS):
                        if "m" in phases:
                            emit_msg_window(w)
                        if "g" in phases and w % WG == WG - 1:
                            emit_gru_group(w - WG + 1)

            if repeats > 1 and hw_loop:
                with tc.For_i(0, repeats):
                    emit_eval()
            else:
                for _rep in range(repeats):
                    emit_eval()

    nc.compile()
    return nc


# message-column permutation: evens first, then odds
PERM = np.concatenate([np.arange(0, TWO_MSG, 2), np.arange(1, TWO_MSG, 2)])


def _prep_host(feat, h, src, dst, u, dg_bias=None):
    """Host-side sharding/layout. Returns (B, list of per-core in_maps)."""
    import ml_dtypes
    bf16 = ml_dtypes.bfloat16
    fp8 = ml_dtypes.float8_e4m3

    feat = np.ascontiguousarray(feat, dtype=np.float32)
    h = np.ascontiguousarray(h, dtype=np.float32)
    src = np.asarray(src).astype(np.int64)
    dst = np.asarray(dst).astype(np.int64)
    u = np.asarray(u, dtype=np.float32)

    featH = np.concatenate([feat, h], axis=1)  # [N, 256]
    featH_pad = np.zeros((ZPAD, 256), np.float32)
    featH_pad[:N_NODES] = featH
    # channel-major fp8: [p, zb, a, n] = featH_pad[zb*128+n, a*128+p]
    fh8 = np.ascontiguousarray(
        featH_pad.astype(fp8).reshape(ZBLKS, 128, 2, 128).transpose(3, 0, 2, 1)
    ).reshape(128, -1)

    # host Gumbel difference, folding b_enc (b_enc=0 here but stay general at
    # the call site via _prep_weights -> dg_bias)
    dgf = (np.log(-np.log(u[..., 0].astype(np.float64) + EPS) + EPS)
           - np.log(-np.log(u[..., 1].astype(np.float64) + EPS) + EPS))
    if dg_bias is not None:
        dgf = dgf - np.asarray(dg_bias, np.float64)[None, :]

    order = np.argsort(dst, kind="stable")
    dst_s = dst[order]
    src_s = src[order]
    win = dst_s // WIN_NODES                     # window id per sorted edge
    counts = np.bincount(win, minlength=N_WINDOWS)
    starts = np.zeros(N_WINDOWS + 1, np.int64)
    np.cumsum(counts, out=starts[1:])
    B = int(np.max((counts + 127) // 128))
    B = max(B, 1)
    EW = B * 128
    EDEV = WINS * EW

    in_maps = []
    for d in range(NDEV):
        src_pad = np.zeros((EDEV,), np.int64)
        slot_pad = np.full((EDEV,), -1.0, np.float32)
        dg_pad = np.zeros((EDEV, MSG), np.float32)
        for k in range(WINS):
            wid = d * WINS + k
            s, e = starts[wid], starts[wid + 1]
            n = e - s
            o = k * EW
            src_pad[o : o + n] = src_s[s:e]
            slot_pad[o : o + n] = (dst_s[s:e] - wid * WIN_NODES).astype(np.float32)
            dg_pad[o : o + n] = dgf[order[s:e]]

        # partition-major table index: row of node n is (n%128)*ZBLKS + n//128
        srcT = (src_pad % 128) * ZBLKS + src_pad // 128
        # gather idx layout: [p, s] = idx[16*s + p%16], replicated across groups
        idx16 = np.empty((128, EDEV // 16), np.int16)
        flat = srcT.astype(np.int16).reshape(EDEV // 16, 16).T  # [16, EDEV/16]
        for g in range(8):
            idx16[g * 16 : (g + 1) * 16, :] = flat
        # compact dstslot: [p, w*B + b] = slot of edge (w, b, p)
        dstslot_c = np.ascontiguousarray(
            slot_pad.reshape(WINS * B, 128).T.astype(bf16))
        # dg swizzled: [p, blk*64 + c] = dg_pad[blk*128 + p, c]
        dg_sw = np.ascontiguousarray(
            dg_pad.astype(fp8).reshape(EDEV // 128, 128, MSG)
            .transpose(1, 0, 2)).reshape(128, -1)
        # local featH channel-major bf16: [p, w, a, n] = featH[node, a*128+p]
        base = d * DEV_NODES
        end = min(base + DEV_NODES, N_NODES)
        locflat = np.zeros((DEV_NODES, 256), np.float32)
        locflat[: end - base] = featH[base:end]
        loc = np.zeros((WINS, 128, 2, 128), np.float32)  # [w, n, a, p]
        loc[:, :WIN_NODES] = locflat.reshape(WINS, WIN_NODES, 2, 128)
        fh_locT = np.ascontiguousarray(
            loc.astype(bf16).transpose(3, 0, 2, 1)).reshape(128, -1)
        h_pad = np.zeros((DEV_NODES, HIDDEN), np.float32)
        h_pad[: end - base] = h[base:end]
        h_loc = np.ascontiguousarray(h_pad.astype(bf16))
        in_maps.append({
            "fh8": fh8, "fh_locT": fh_locT, "h_loc": h_loc,
            "dg_g": dg_sw, "src16": idx16, "dstslot": dstslot_c,
        })
    return B, in_maps


def _prep_weights(W_enc, b_enc, W_dec, b_dec, W_ih, W_hh, b_ih, b_hh):
    import ml_dtypes
    bf16 = ml_dtypes.bfloat16
    fp8 = ml_dtypes.float8_e4m3

    W_enc = np.asarray(W_enc, np.float32)
    W_dec = np.asarray(W_dec, np.float32)
    W_ih = np.asarray(W_ih, np.float32)
    W_hh = np.asarray(W_hh, np.float32)
    b_enc = np.asarray(b_enc, np.float32)
    b_dec = np.asarray(b_dec, np.float32)
    b_ih = np.asarray(b_ih, np.float32)
    b_hh = np.asarray(b_hh, np.float32)

    Wd = W_enc[0::2].astype(np.float64) - W_enc[1::2].astype(np.float64)  # [64, 256]
    wd8T = np.ascontiguousarray(Wd.T.astype(fp8))            # [256, 64]

    W_b = (W_ih[:, HIDDEN:].astype(np.float64) @ W_dec.astype(np.float64))
    b_comb = (W_ih[:, HIDDEN:].astype(np.float64) @ b_dec.astype(np.float64)) + b_ih

    waT = np.ascontiguousarray(W_ih[:, :HIDDEN].T.astype(bf16))      # [128, 384]
    wbT = np.ascontiguousarray(W_b.T.astype(np.float32)[PERM, :].astype(bf16))
    whhT = np.ascontiguousarray(W_hh.T.astype(bf16))                 # [128, 384]
    brz = (b_comb[:256] + b_hh[:256]).astype(np.float32)
    bn = b_comb[256:384].astype(np.float32)
    bhn = b_hh[256:384].astype(np.float32)
    return {
        "wd8T": wd8T,
        "waT": waT, "wbT": wbT, "whhT": whhT,
        "bias_rz": np.ascontiguousarray(np.tile(brz, (128, 1))),
        "bias_n": np.ascontiguousarray(np.tile(bn, (128, 1))),
        "bias_hn": np.ascontiguousarray(np.tile(bhn, (128, 1))),
    }, (b_enc[0::2].astype(np.float64) - b_enc[1::2].astype(np.float64))


def kernel(feat, h, src, dst, u, W_enc, b_enc, W_dec, b_dec, W_ih, W_hh,
           b_ih, b_hh):
    wmap, dg_bias = _prep_weights(W_enc, b_enc, W_dec, b_dec, W_ih, W_hh,
                                  b_ih, b_hh)
    B, in_maps = _prep_host(feat, h, src, dst, u, dg_bias=dg_bias)
    for m in in_maps:
        m.update(wmap)

    phases = os.environ.get("KERNEL_PHASES", "zmg")
    zero_bias = not (np.any(np.asarray(b_dec)) or np.any(np.asarray(b_ih))
                     or np.any(np.asarray(b_hh)))
    key = (B, phases, zero_bias)
    if key not in _cache:
        _cache[key] = build_program(B, phases, zero_bias)
    nc = _cache[key]

    res = run_bass_kernel_spmd(nc, in_maps, core_ids=list(range(NDEV)))
    h_new = np.concatenate([res.results[d]["h_new"] for d in range(NDEV)],
                           axis=0)[:N_NODES]
    return (h_new, h_new)



# revision 31
# speedup vs baseline: 5.6017x; 1.2859x over previous
"""Trainium2 Bass kernel for nn_DiscreteCommunication (GNN message passing).

v2 strategy (8 NeuronCores, SPMD single program, no collectives).

Key reduction: with hard=True straight-through Gumbel-softmax over 2 options,
the forward message is exactly one-hot, so only sign(z0 - z1 + g0 - g1)
matters. Define zd = featH @ (W_enc[evens] - W_enc[odds]).T  (64 cols) and
dg = ln(-ln(u0)+e) - ln(-ln(u1)+e) - (b_enc[evens]-b_enc[odds]) (host-
precomputed, bf16). Then m_even = (zd[src] >= dg); m_odd = 1 - m_even.

  - Host: sort edges by dst; device d owns dst nodes [2500d, 2500d+2500) as
    20 windows of 125 nodes. Edges padded to B 128-edge blocks per window
    (same B on all devices -> one SPMD program).
  - Phase Z: zd = featH_fp8 @ Wd_fp8.T over all 20096 padded nodes ->
    Zd table in local DRAM, bf16 rows padded to 128 cols (gather rows must
    be a multiple of 256B). Table is partition-major (row r = (n%128)*157 +
    n//128) so the table write is contiguous per partition; the gather
    index is host-transformed to match.
  - Phase MSG per window: dma_gather Zd[src'] (256B rows), m0 = (zg >= dg)
    bf16 (2x DVE), m1 = 1-m0, one-hot P = (iota == dstslot), segment-sum
    via PE: cps += m_b.T @ P_b, c = sign(cps) in bf16.
  - Phase GRU: node-parallel, all-bf16 matmuls (4x faster than f32 on PE),
    f32 elementwise/blend. dec folded: gi = feat@W_a.T + c@(W_ih[:,128:]
    @W_dec).T.
  - Host: concatenate the 8 per-device h_new slices.
"""
import os
import sys

sys.path.insert(0, "/opt/trn_rl_repo")

import numpy as np
import concourse.bacc as bacc
import concourse.mybir as mybir
import concourse.tile as tile
from concourse.bass_utils import run_bass_kernel_spmd

F32 = mybir.dt.float32
BF16 = mybir.dt.bfloat16
FP8 = mybir.dt.float8e4
I16 = mybir.dt.int16
AF = mybir.ActivationFunctionType
OP = mybir.AluOpType

N_NODES = 20000
HIDDEN = 128
MSG = 64
TWO_MSG = 2 * MSG  # 128
N_EDGES = 320000
EPS = 1e-10
NDEV = 8
WIN_NODES = 125            # nodes per window (<=128 for one-hot slots)
WINS = 20                  # windows per device
DEV_NODES = WIN_NODES * WINS   # 2500
N_WINDOWS = NDEV * WINS        # 160, covers all 20000 nodes exactly
ZBLKS = (N_NODES + 127) // 128  # 157 blocks over nodes (last partial: 32)
ZPAD = ZBLKS * 128              # 20096
GCH = 1024                 # gather call chunk (hard cap: 64 desc/engine/call)
QW = 32                    # one-hot slot-quarter width for the scatter matmul
SCRATCH = 49152                 # SWDGE ring: 3072 descriptors

_cache = {}


def build_program(B, qranges=None, phases="zmg", zero_bias=True, repeats=1,
                  skip_gather=False, skip_zmm=False, skip_zwrite=False,
                  skip_msgmm=False, strided_zwrite=True, zg_group=16,
                  hw_loop=True, loop_unroll=4):
    """Build the SPMD Bass program for B blocks-per-window.

    qranges: tuple of WINS tuples of 4 (qb0, qb1) block ranges — for each
    window and slot-quarter q, the [qb0, qb1) range of 128-edge blocks that
    contain edges whose dstslot is in [QW*q, QW*(q+1)). Derived from the data
    (max over devices) so it is SPMD-uniform. None -> full range for each q.
    """
    if qranges is None:
        qranges = tuple(tuple((0, B) for _ in range(4)) for _ in range(WINS))
    NQB = [sum(b1 - b0 for b0, b1 in qranges[w]) for w in range(WINS)]
    NQBmax = max(NQB)
    qoff_tab = []  # per window: col offset of each quarter in dslotq
    off = 0
    for w in range(WINS):
        offs = []
        for q in range(4):
            offs.append(off)
            off += qranges[w][q][1] - qranges[w][q][0]
        qoff_tab.append(offs)
    NQB_TOT = off
    nc = bacc.Bacc("TRN2", target_bir_lowering=False,
                   dynamic_dma_scratch_size=SCRATCH, num_swdge_queues=4)
    EW = B * 128               # padded edges per window
    EDEV = WINS * EW           # padded edges per device

    # ---- I/O ----
    # channel-major featH fp8: [p, zb, a, n] = featH_pad[zb*128+n, a*128+p]
    fh8 = nc.dram_tensor("fh8", [128, ZBLKS * 256], FP8, kind="ExternalInput")
    fh_locT = nc.dram_tensor("fh_locT", [128, WINS * 256], BF16, kind="ExternalInput")
    h_loc = nc.dram_tensor("h_loc", [DEV_NODES, HIDDEN], BF16, kind="ExternalInput")
    dg_g = nc.dram_tensor("dg_g", [128, WINS * B * MSG], FP8, kind="ExternalInput")
    src16 = nc.dram_tensor("src16", [128, EDEV // 16], I16, kind="ExternalInput")
    dstslot = nc.dram_tensor("dstslot", [128, NQB_TOT], BF16, kind="ExternalInput")
    wd8T = nc.dram_tensor("wd8T", [256, MSG], FP8, kind="ExternalInput")
    waT = nc.dram_tensor("waT", [128, 384], BF16, kind="ExternalInput")
    wbT = nc.dram_tensor("wbT", [128, 384], BF16, kind="ExternalInput")
    whhT = nc.dram_tensor("whhT", [128, 384], BF16, kind="ExternalInput")
    if not zero_bias:
        bias_rz = nc.dram_tensor("bias_rz", [128, 256], F32, kind="ExternalInput")
        bias_n = nc.dram_tensor("bias_n", [128, 128], F32, kind="ExternalInput")
        bias_hn = nc.dram_tensor("bias_hn", [128, 128], F32, kind="ExternalInput")
    h_new = nc.dram_tensor("h_new", [DEV_NODES, HIDDEN], BF16, kind="ExternalOutput")

    with tile.TileContext(nc) as tc:
        with (
            tc.tile_pool(name="const", bufs=1) as cp,
        ):
            # ---- persistent constants ----
            iota32 = cp.tile([128, NQBmax, QW], BF16)
            # values 0..QW-1 repeated per quarter-block; exact in bf16
            nc.gpsimd.iota(iota32[:], pattern=[[0, NQBmax], [1, QW]], base=0,
                           channel_multiplier=0,
                           allow_small_or_imprecise_dtypes=True)
            dslot_t = cp.tile([128, NQB_TOT], BF16)
            nc.scalar.dma_start(out=dslot_t[:], in_=dstslot[:])
            wd_t = cp.tile([128, 2, MSG], FP8)
            nc.sync.dma_start(out=wd_t[:], in_=wd8T.rearrange("(a p) j -> p a j", p=128))
            waT_t = cp.tile([128, 384], BF16)
            nc.sync.dma_start(out=waT_t[:], in_=waT[:])
            wbT_t = cp.tile([128, 384], BF16)
            nc.sync.dma_start(out=wbT_t[:], in_=wbT[:])
            whhT_t = cp.tile([128, 384], BF16)
            nc.sync.dma_start(out=whhT_t[:], in_=whhT[:])
            src16_t = cp.tile([128, EDEV // 16], I16)
            # biggest constant (0.7MB): keep it off the sync queue so it
            # overlaps the first fh8 feature loads in a single-eval launch
            nc.scalar.dma_start(out=src16_t[:], in_=src16[:])
            if not zero_bias:
                bias_rz_t = cp.tile([128, 256], F32)
                nc.sync.dma_start(out=bias_rz_t[:], in_=bias_rz[:])
                bias_n_t = cp.tile([128, 128], F32)
                nc.sync.dma_start(out=bias_n_t[:], in_=bias_n[:])
                bias_hn_t = cp.tile([128, 128], F32)
                nc.sync.dma_start(out=bias_hn_t[:], in_=bias_hn[:])
            cT_tiles = []
            for w in range(WINS):
                ct = cp.tile([128, 128], BF16, tag=f"cT{w}")
                cT_tiles.append(ct)

            # Zd table: row r=(n%128)*ZBLKS + n//128 (partition-major), 256
            # fp8 cols (0:64 = zd, 64:256 junk pad to reach the 256B-row
            # minimum of dma_gather).
            ZROW = 256
            Zd = nc.dram_tensor("Zd", [ZPAD, ZROW], FP8)
            Zdv = Zd.rearrange("(p g) j -> p (g j)", p=128)  # [128, ZBLKS*256]

            # ---- Phase Z: zd = featH_fp8 @ Wd_fp8.T (all nodes) ----
            ZG = zg_group
            def emit_z_phase():
             with (
                tc.tile_pool(name="zio", bufs=4) as zio,
                tc.tile_pool(name="zps", bufs=3, space="PSUM") as zps,
             ):
              engs = [nc.sync, nc.scalar]
              for gi_, g0 in enumerate(range(0, ZBLKS, ZG)):
                gn = min(ZG, ZBLKS - g0)
                fg = zio.tile([128, ZG, 2, 128], FP8, tag="fg")
                cols = gn * 256
                engs[gi_ % 2].dma_start(
                    out=fg[:].rearrange("p g a n -> p (g a n)")[:, :cols],
                    in_=fh8[:, g0 * 256 : g0 * 256 + cols])
                zp = zps.tile([128, ZG * MSG], F32, space="PSUM", tag="zp")
                if not skip_zmm:
                    for zi in range(gn):
                        zslc = zp[:, zi * MSG : (zi + 1) * MSG]
                        nc.tensor.matmul(out=zslc, lhsT=fg[:, zi, 0, :],
                                         rhs=wd_t[:, 0, :], start=True, stop=False)
                        nc.tensor.matmul(out=zslc, lhsT=fg[:, zi, 1, :],
                                         rhs=wd_t[:, 1, :], start=False, stop=True)
                else:
                    nc.vector.memset(zp[:, : gn * MSG], 0.0)
                zs = zio.tile([128, ZG, MSG], FP8, tag="zs")
                nc.scalar.copy(
                    out=zs[:, :gn, :],
                    in_=zp[:, : gn * MSG].rearrange("p (g j) -> p g j", g=gn))
                if not skip_zwrite:
                    weng = engs[(gi_ + 1) % 2]
                    weng.dma_start(
                        out=Zd.rearrange("(p g) j -> p g j", p=128)[:, g0 : g0 + gn, 0:MSG],
                        in_=zs[:, :gn, :])

            # ---- Phase MSG + GRU, interleaved ----
            WG = 4
            qctr = [0]
            mp = pp = gp = mps = rp = rps = rps2 = None
            def emit_msg_window(w):
                # P one-hot first: no gather dependency, so the in-order DVE
                # queue builds it while this window's gather is in flight
                nqb_w = NQB[w]
                woff = qoff_tab[w][0]
                P = pp.tile([128, NQBmax, QW], FP8, tag="P")
                nc.vector.tensor_tensor(
                    out=P[:, :nqb_w, :],
                    in0=iota32[:, :nqb_w, :],
                    in1=dslot_t[:, woff : woff + nqb_w, None]
                        .to_broadcast([128, nqb_w, QW]),
                    op=OP.is_equal)
                zg = gp.tile([128, B, ZROW], FP8, tag="zg")
                if skip_gather:
                    nc.vector.memset(zg[:], 0.0)
                else:
                    off = 0
                    while off < EW:
                        chunk = min(GCH, EW - off)
                        nc.gpsimd.dma_gather(
                            zg[:, off // 128 : (off + chunk) // 128, :], Zd[:],
                            src16_t[:, (w * EW + off) // 16 : (w * EW + off + chunk) // 16],
                            num_idxs=chunk, num_idxs_reg=chunk, elem_size=ZROW,
                            queue_num=qctr[0] % 4,
                        )
                        qctr[0] += 1
                        off += chunk
                dgw = gp.tile([128, B, MSG], FP8, tag="dgw")
                ueng = nc.sync if w % 2 == 0 else nc.scalar
                ueng.dma_start(
                    out=dgw[:].rearrange("p b c -> p (b c)"),
                    in_=dg_g[:, w * B * MSG : (w + 1) * B * MSG])
                m = mp.tile([128, B, TWO_MSG], FP8, tag="m")
                nc.vector.tensor_tensor(out=m[:, :, 0:MSG], in0=zg[:, :, 0:MSG],
                                        in1=dgw[:], op=OP.is_ge)
                # m1 = 1 - m0 == (m0 < 1)
                nc.vector.tensor_scalar(out=m[:, :, MSG:TWO_MSG], in0=m[:, :, 0:MSG],
                                        scalar1=1.0, scalar2=None, op0=OP.is_lt)
                cps = mps.tile([128, 128], F32, space="PSUM", tag="cps")
                for q in range(4):
                    qb0, qb1 = qranges[w][q]
                    creg = cps[:, QW * q : QW * (q + 1)]
                    if qb1 <= qb0 or skip_msgmm:
                        nc.vector.memset(creg, 0.0)
                        continue
                    po = qoff_tab[w][q] - woff
                    for b in range(qb0, qb1):
                        nc.tensor.matmul(out=creg, lhsT=m[:, b, :],
                                         rhs=P[:, po + (b - qb0), :],
                                         start=(b == qb0), stop=(b == qb1 - 1))
                # c = (c_sum > 0) == Sign(c_sum) since c_sum >= 0; runs on ACT
                nc.scalar.sign(out=cT_tiles[w][:], in_=cps[:])

            def emit_gru_group(w0):
                xh = rp.tile([128, WG, 2, 128], BF16, tag="xh")
                nc.sync.dma_start(
                    out=xh[:].rearrange("p w a n -> p (w a n)"),
                    in_=fh_locT[:, w0 * 256 : (w0 + WG) * 256])
                hl = rp.tile([128, WG, 128], BF16, tag="hl")
                for wi in range(WG):
                    w = w0 + wi
                    nc.scalar.dma_start(
                        out=hl[:WIN_NODES, wi, :],
                        in_=h_loc[w * WIN_NODES : (w + 1) * WIN_NODES, :])
                # two PSUM groups per window (rz gates / n gate) so each
                # group is opened and closed over exactly the same col range
                gi = rps.tile([128, WG, 256], F32, space="PSUM", tag="gi")
                gn_ps = rps.tile([128, WG, 128], F32, space="PSUM", tag="gn_ps")
                hn_ps = rps2.tile([128, WG, 128], F32, space="PSUM", tag="hn_ps")
                for wi in range(WG):
                    w = w0 + wi
                    nc.tensor.matmul(out=gi[:, wi, :], lhsT=xh[:, wi, 0, :],
                                     rhs=waT_t[:, 0:256], start=True, stop=False)
                    nc.tensor.matmul(out=gi[:, wi, :], lhsT=cT_tiles[w][:],
                                     rhs=wbT_t[:, 0:256], start=False, stop=False)
                    nc.tensor.matmul(out=gi[:, wi, :], lhsT=xh[:, wi, 1, :],
                                     rhs=whhT_t[:, 0:256], start=False, stop=True)
                    nc.tensor.matmul(out=gn_ps[:, wi, :], lhsT=xh[:, wi, 0, :],
                                     rhs=waT_t[:, 256:384], start=True, stop=False)
                    nc.tensor.matmul(out=gn_ps[:, wi, :], lhsT=cT_tiles[w][:],
                                     rhs=wbT_t[:, 256:384], start=False, stop=True)
                    nc.tensor.matmul(out=hn_ps[:, wi, :], lhsT=xh[:, wi, 1, :],
                                     rhs=whhT_t[:, 256:384], start=True, stop=True)
                V = WIN_NODES
                rz_s = rp.tile([128, WG, 256], F32, tag="rz_s")
                if zero_bias:
                    nc.scalar.activation(rz_s[:V], gi[:V], AF.Sigmoid)
                    rhn = rp.tile([128, WG, 128], F32, tag="rhn")
                    nc.vector.tensor_tensor(out=rhn[:V], in0=rz_s[:V, :, 0:128],
                                            in1=hn_ps[:V], op=OP.mult)
                    narg = rp.tile([128, WG, 128], F32, tag="narg")
                    nc.vector.tensor_tensor(out=narg[:V], in0=rhn[:V],
                                            in1=gn_ps[:V], op=OP.add)
                else:
                    rz = rp.tile([128, WG, 256], F32, tag="rz")
                    nc.vector.tensor_tensor(
                        out=rz[:V], in0=gi[:V],
                        in1=bias_rz_t[:V, None, :].to_broadcast([V, WG, 256]), op=OP.add)
                    nc.scalar.activation(rz_s[:V], rz[:V], AF.Sigmoid)
                    hn = rp.tile([128, WG, 128], F32, tag="hn")
                    nc.vector.tensor_tensor(
                        out=hn[:V], in0=hn_ps[:V],
                        in1=bias_hn_t[:V, None, :].to_broadcast([V, WG, 128]), op=OP.add)
                    inn = rp.tile([128, WG, 128], F32, tag="inn")
                    nc.vector.tensor_tensor(
                        out=inn[:V], in0=gn_ps[:V],
                        in1=bias_n_t[:V, None, :].to_broadcast([V, WG, 128]), op=OP.add)
                    rhn = rp.tile([128, WG, 128], F32, tag="rhn")
                    nc.vector.tensor_tensor(out=rhn[:V], in0=rz_s[:V, :, 0:128], in1=hn[:V], op=OP.mult)
                    narg = rp.tile([128, WG, 128], F32, tag="narg")
                    nc.vector.tensor_tensor(out=narg[:V], in0=inn[:V], in1=rhn[:V], op=OP.add)
                n_t = rp.tile([128, WG, 128], F32, tag="n_t")
                nc.scalar.activation(n_t[:V], narg[:V], AF.Tanh)
                hmn = rp.tile([128, WG, 128], F32, tag="hmn")
                nc.vector.tensor_tensor(out=hmn[:V], in0=hl[:V], in1=n_t[:V], op=OP.subtract)
                zh = rp.tile([128, WG, 128], F32, tag="zh")
                nc.vector.tensor_tensor(out=zh[:V], in0=rz_s[:V, :, 128:256], in1=hmn[:V], op=OP.mult)
                ho = rp.tile([128, WG, 128], BF16, tag="ho")
                nc.vector.tensor_tensor(out=ho[:V], in0=n_t[:V], in1=zh[:V], op=OP.add)
                for wi in range(WG):
                    w = w0 + wi
                    nc.sync.dma_start(
                        out=h_new[w * WIN_NODES : (w + 1) * WIN_NODES, :],
                        in_=ho[:WIN_NODES, wi, :])

            def emit_eval():
                nonlocal mp, pp, gp, mps, rp, rps, rps2
                if "z" in phases:
                    emit_z_phase()
                with (
                    tc.tile_pool(name="msg", bufs=2) as mp,
                    tc.tile_pool(name="ponehot", bufs=3) as pp,
                    tc.tile_pool(name="gat", bufs=3) as gp,
                    tc.tile_pool(name="mps", bufs=2, space="PSUM") as mps,
                    tc.tile_pool(name="gru", bufs=2) as rp,
                    tc.tile_pool(name="rps", bufs=1, space="PSUM") as rps,
                    tc.tile_pool(name="rps2", bufs=2, space="PSUM") as rps2,
                ):
                    for w in range(WINS):
                        if "m" in phases:
                            emit_msg_window(w)
                        if "g" in phases and w % WG == WG - 1:
                            emit_gru_group(w - WG + 1)

            if repeats > 1 and hw_loop:
                U = loop_unroll
                assert repeats % U == 0
                with tc.For_i(0, repeats // U):
                    for _u in range(U):
                        emit_eval()
            else:
                for _rep in range(repeats):
                    emit_eval()

    nc.compile()
    return nc


# message-column permutation: evens first, then odds
PERM = np.concatenate([np.arange(0, TWO_MSG, 2), np.arange(1, TWO_MSG, 2)])


def _prep_host(feat, h, src, dst, u, dg_bias=None):
    """Host-side sharding/layout. Returns (B, list of per-core in_maps)."""
    import ml_dtypes
    bf16 = ml_dtypes.bfloat16
    fp8 = ml_dtypes.float8_e4m3

    feat = np.ascontiguousarray(feat, dtype=np.float32)
    h = np.ascontiguousarray(h, dtype=np.float32)
    src = np.asarray(src).astype(np.int64)
    dst = np.asarray(dst).astype(np.int64)
    u = np.asarray(u, dtype=np.float32)

    featH = np.concatenate([feat, h], axis=1)  # [N, 256]
    featH_pad = np.zeros((ZPAD, 256), np.float32)
    featH_pad[:N_NODES] = featH
    # channel-major fp8: [p, zb, a, n] = featH_pad[zb*128+n, a*128+p]
    fh8 = np.ascontiguousarray(
        featH_pad.astype(fp8).reshape(ZBLKS, 128, 2, 128).transpose(3, 0, 2, 1)
    ).reshape(128, -1)

    # host Gumbel difference, folding b_enc (b_enc=0 here but stay general at
    # the call site via _prep_weights -> dg_bias)
    dgf = (np.log(-np.log(u[..., 0].astype(np.float64) + EPS) + EPS)
           - np.log(-np.log(u[..., 1].astype(np.float64) + EPS) + EPS))
    if dg_bias is not None:
        dgf = dgf - np.asarray(dg_bias, np.float64)[None, :]

    order = np.argsort(dst, kind="stable")
    dst_s = dst[order]
    src_s = src[order]
    win = dst_s // WIN_NODES                     # window id per sorted edge
    counts = np.bincount(win, minlength=N_WINDOWS)
    starts = np.zeros(N_WINDOWS + 1, np.int64)
    np.cumsum(counts, out=starts[1:])
    B = int(np.max((counts + 127) // 128))
    B = max(B, 1)
    EW = B * 128
    EDEV = WINS * EW

    # per (window-in-device, quarter): block range [qb0, qb1) containing any
    # edge with dstslot in [QW*q, QW*(q+1)); SPMD-uniform = min/max over devs
    qb0 = np.full((NDEV, WINS, 4), B, np.int64)
    qb1 = np.zeros((NDEV, WINS, 4), np.int64)
    for wid in range(N_WINDOWS):
        d, k = divmod(wid, WINS)
        s, e = starts[wid], starts[wid + 1]
        slots = dst_s[s:e] - wid * WIN_NODES     # ascending within window
        qidx = slots // QW
        pos = np.arange(e - s)
        for q in range(4):
            sel = pos[qidx == q]
            if len(sel):
                qb0[d, k, q] = sel[0] // 128
                qb1[d, k, q] = sel[-1] // 128 + 1
    qb0g = qb0.min(axis=0)
    qb1g = qb1.max(axis=0)
    empty = qb1g <= qb0g
    qb0g[empty] = 0
    qb1g[empty] = 0
    qranges = tuple(tuple((int(qb0g[k, q]), int(qb1g[k, q])) for q in range(4))
                    for k in range(WINS))

    in_maps = []
    for d in range(NDEV):
        src_pad = np.zeros((EDEV,), np.int64)
        slot_pad = np.full((EDEV,), -1.0, np.float32)
        dg_pad = np.zeros((EDEV, MSG), np.float32)
        for k in range(WINS):
            wid = d * WINS + k
            s, e = starts[wid], starts[wid + 1]
            n = e - s
            o = k * EW
            src_pad[o : o + n] = src_s[s:e]
            slot_pad[o : o + n] = (dst_s[s:e] - wid * WIN_NODES).astype(np.float32)
            dg_pad[o : o + n] = dgf[order[s:e]]

        # partition-major table index: row of node n is (n%128)*ZBLKS + n//128
        srcT = (src_pad % 128) * ZBLKS + src_pad // 128
        # gather idx layout: [p, s] = idx[16*s + p%16], replicated across groups
        idx16 = np.empty((128, EDEV // 16), np.int16)
        flat = srcT.astype(np.int16).reshape(EDEV // 16, 16).T  # [16, EDEV/16]
        for g in range(8):
            idx16[g * 16 : (g + 1) * 16, :] = flat
        # quarter-shifted compact dstslot: one 128-row per (w, q, b-in-range)
        # col; value = slot - QW*q (values outside [0, QW) match no one-hot)
        slot_w = slot_pad.reshape(WINS, B, 128)
        cols = []
        for k in range(WINS):
            for q in range(4):
                b0, b1 = qranges[k][q]
                if b1 > b0:
                    cols.append(slot_w[k, b0:b1, :] - QW * q)
        dstslot_c = np.ascontiguousarray(
            np.concatenate(cols, axis=0).T.astype(bf16))  # [128, NQB_TOT]
        # dg swizzled: [p, blk*64 + c] = dg_pad[blk*128 + p, c]
        dg_sw = np.ascontiguousarray(
            dg_pad.astype(fp8).reshape(EDEV // 128, 128, MSG)
            .transpose(1, 0, 2)).reshape(128, -1)
        # local featH channel-major bf16: [p, w, a, n] = featH[node, a*128+p]
        base = d * DEV_NODES
        end = min(base + DEV_NODES, N_NODES)
        locflat = np.zeros((DEV_NODES, 256), np.float32)
        locflat[: end - base] = featH[base:end]
        loc = np.zeros((WINS, 128, 2, 128), np.float32)  # [w, n, a, p]
        loc[:, :WIN_NODES] = locflat.reshape(WINS, WIN_NODES, 2, 128)
        fh_locT = np.ascontiguousarray(
            loc.astype(bf16).transpose(3, 0, 2, 1)).reshape(128, -1)
        h_pad = np.zeros((DEV_NODES, HIDDEN), np.float32)
        h_pad[: end - base] = h[base:end]
        h_loc = np.ascontiguousarray(h_pad.astype(bf16))
        in_maps.append({
            "fh8": fh8, "fh_locT": fh_locT, "h_loc": h_loc,
            "dg_g": dg_sw, "src16": idx16, "dstslot": dstslot_c,
        })
    return B, qranges, in_maps


def _prep_weights(W_enc, b_enc, W_dec, b_dec, W_ih, W_hh, b_ih, b_hh):
    import ml_dtypes
    bf16 = ml_dtypes.bfloat16
    fp8 = ml_dtypes.float8_e4m3

    W_enc = np.asarray(W_enc, np.float32)
    W_dec = np.asarray(W_dec, np.float32)
    W_ih = np.asarray(W_ih, np.float32)
    W_hh = np.asarray(W_hh, np.float32)
    b_enc = np.asarray(b_enc, np.float32)
    b_dec = np.asarray(b_dec, np.float32)
    b_ih = np.asarray(b_ih, np.float32)
    b_hh = np.asarray(b_hh, np.float32)

    Wd = W_enc[0::2].astype(np.float64) - W_enc[1::2].astype(np.float64)  # [64, 256]
    wd8T = np.ascontiguousarray(Wd.T.astype(fp8))            # [256, 64]

    W_b = (W_ih[:, HIDDEN:].astype(np.float64) @ W_dec.astype(np.float64))
    b_comb = (W_ih[:, HIDDEN:].astype(np.float64) @ b_dec.astype(np.float64)) + b_ih

    waT = np.ascontiguousarray(W_ih[:, :HIDDEN].T.astype(bf16))      # [128, 384]
    wbT = np.ascontiguousarray(W_b.T.astype(np.float32)[PERM, :].astype(bf16))
    whhT = np.ascontiguousarray(W_hh.T.astype(bf16))                 # [128, 384]
    brz = (b_comb[:256] + b_hh[:256]).astype(np.float32)
    bn = b_comb[256:384].astype(np.float32)
    bhn = b_hh[256:384].astype(np.float32)
    return {
        "wd8T": wd8T,
        "waT": waT, "wbT": wbT, "whhT": whhT,
        "bias_rz": np.ascontiguousarray(np.tile(brz, (128, 1))),
        "bias_n": np.ascontiguousarray(np.tile(bn, (128, 1))),
        "bias_hn": np.ascontiguousarray(np.tile(bhn, (128, 1))),
    }, (b_enc[0::2].astype(np.float64) - b_enc[1::2].astype(np.float64))


def kernel(feat, h, src, dst, u, W_enc, b_enc, W_dec, b_dec, W_ih, W_hh,
           b_ih, b_hh):
    wmap, dg_bias = _prep_weights(W_enc, b_enc, W_dec, b_dec, W_ih, W_hh,
                                  b_ih, b_hh)
    B, qranges, in_maps = _prep_host(feat, h, src, dst, u, dg_bias=dg_bias)
    for m in in_maps:
        m.update(wmap)

    phases = os.environ.get("KERNEL_PHASES", "zmg")
    zero_bias = not (np.any(np.asarray(b_dec)) or np.any(np.asarray(b_ih))
                     or np.any(np.asarray(b_hh)))
    key = (B, qranges, phases, zero_bias)
    if key not in _cache:
        _cache[key] = build_program(B, qranges, phases, zero_bias)
    nc = _cache[key]

    res = run_bass_kernel_spmd(nc, in_maps, core_ids=list(range(NDEV)))
    h_new = np.concatenate([res.results[d]["h_new"] for d in range(NDEV)],
                           axis=0)[:N_NODES].astype(np.float32)
    return (h_new, h_new)



# revision 34
# speedup vs baseline: 6.0466x; 1.0794x over previous
"""Trainium2 Bass kernel for nn_DiscreteCommunication (GNN message passing).

v2 strategy (8 NeuronCores, SPMD single program, no collectives).

Key reduction: with hard=True straight-through Gumbel-softmax over 2 options,
the forward message is exactly one-hot, so only sign(z0 - z1 + g0 - g1)
matters. Define zd = featH @ (W_enc[evens] - W_enc[odds]).T  (64 cols) and
dg = ln(-ln(u0)+e) - ln(-ln(u1)+e) - (b_enc[evens]-b_enc[odds]) (host-
precomputed, bf16). Then m_even = (zd[src] >= dg); m_odd = 1 - m_even.

  - Host: sort edges by dst; device d owns dst nodes [2500d, 2500d+2500) as
    20 windows of 125 nodes. Edges padded to B 128-edge blocks per window
    (same B on all devices -> one SPMD program).
  - Phase Z: zd = featH_fp8 @ Wd_fp8.T over all 20096 padded nodes ->
    Zd table in local DRAM, bf16 rows padded to 128 cols (gather rows must
    be a multiple of 256B). Table is partition-major (row r = (n%128)*157 +
    n//128) so the table write is contiguous per partition; the gather
    index is host-transformed to match.
  - Phase MSG per window: dma_gather Zd[src'] (256B rows), m0 = (zg >= dg)
    bf16 (2x DVE), m1 = 1-m0, one-hot P = (iota == dstslot), segment-sum
    via PE: cps += m_b.T @ P_b, c = sign(cps) in bf16.
  - Phase GRU: node-parallel, all-bf16 matmuls (4x faster than f32 on PE),
    f32 elementwise/blend. dec folded: gi = feat@W_a.T + c@(W_ih[:,128:]
    @W_dec).T.
  - Host: concatenate the 8 per-device h_new slices.
"""
import os
import sys

sys.path.insert(0, "/opt/trn_rl_repo")

import numpy as np
import concourse.bacc as bacc
import concourse.mybir as mybir
import concourse.tile as tile
from concourse.bass_utils import run_bass_kernel_spmd

F32 = mybir.dt.float32
BF16 = mybir.dt.bfloat16
FP8 = mybir.dt.float8e4
I16 = mybir.dt.int16
AF = mybir.ActivationFunctionType
OP = mybir.AluOpType

N_NODES = 20000
HIDDEN = 128
MSG = 64
TWO_MSG = 2 * MSG  # 128
N_EDGES = 320000
EPS = 1e-10
NDEV = 8
WIN_NODES = 125            # nodes per window (<=128 for one-hot slots)
WINS = 20                  # windows per device
DEV_NODES = WIN_NODES * WINS   # 2500
N_WINDOWS = NDEV * WINS        # 160, covers all 20000 nodes exactly
ZBLKS = (N_NODES + 127) // 128  # 157 blocks over nodes (last partial: 32)
ZPAD = ZBLKS * 128              # 20096
GCH = 1024                 # gather call chunk (hard cap: 64 desc/engine/call)
QW = 32                    # one-hot slot-quarter width for the scatter matmul
SCRATCH = 49152                 # SWDGE ring: 3072 descriptors

_cache = {}


def build_program(B, qranges=None, phases="zmg", zero_bias=True, repeats=1,
                  skip_gather=False, skip_zmm=False, skip_zwrite=False,
                  skip_msgmm=False, strided_zwrite=True, zg_group=16,
                  hw_loop=True, loop_unroll=8, chunked_compare=False,
                  gru_wg=4):
    """Build the SPMD Bass program for B blocks-per-window.

    qranges: tuple of WINS tuples of 4 (qb0, qb1) block ranges — for each
    window and slot-quarter q, the [qb0, qb1) range of 128-edge blocks that
    contain edges whose dstslot is in [QW*q, QW*(q+1)). Derived from the data
    (max over devices) so it is SPMD-uniform. None -> full range for each q.
    """
    if qranges is None:
        qranges = tuple(tuple((0, B) for _ in range(4)) for _ in range(WINS))
    NQB = [sum(b1 - b0 for b0, b1 in qranges[w]) for w in range(WINS)]
    NQBmax = max(NQB)
    qoff_tab = []  # per window: col offset of each quarter in dslotq
    off = 0
    for w in range(WINS):
        offs = []
        for q in range(4):
            offs.append(off)
            off += qranges[w][q][1] - qranges[w][q][0]
        qoff_tab.append(offs)
    NQB_TOT = off
    nc = bacc.Bacc("TRN2", target_bir_lowering=False,
                   dynamic_dma_scratch_size=SCRATCH, num_swdge_queues=4)
    EW = B * 128               # padded edges per window
    EDEV = WINS * EW           # padded edges per device

    # ---- I/O ----
    # channel-major featH fp8: [p, zb, a, n] = featH_pad[zb*128+n, a*128+p]
    fh8 = nc.dram_tensor("fh8", [128, ZBLKS * 256], FP8, kind="ExternalInput")
    fh_locT = nc.dram_tensor("fh_locT", [128, WINS * 256], BF16, kind="ExternalInput")
    h_loc = nc.dram_tensor("h_loc", [DEV_NODES, HIDDEN], BF16, kind="ExternalInput")
    dg_g = nc.dram_tensor("dg_g", [128, WINS * B * MSG], FP8, kind="ExternalInput")
    src16 = nc.dram_tensor("src16", [128, EDEV // 16], I16, kind="ExternalInput")
    dstslot = nc.dram_tensor("dstslot", [128, NQB_TOT], BF16, kind="ExternalInput")
    wd8T = nc.dram_tensor("wd8T", [256, MSG], FP8, kind="ExternalInput")
    waT = nc.dram_tensor("waT", [128, 384], BF16, kind="ExternalInput")
    wbT = nc.dram_tensor("wbT", [128, 384], BF16, kind="ExternalInput")
    whhT = nc.dram_tensor("whhT", [128, 384], BF16, kind="ExternalInput")
    if not zero_bias:
        bias_rz = nc.dram_tensor("bias_rz", [128, 256], F32, kind="ExternalInput")
        bias_n = nc.dram_tensor("bias_n", [128, 128], F32, kind="ExternalInput")
        bias_hn = nc.dram_tensor("bias_hn", [128, 128], F32, kind="ExternalInput")
    h_new = nc.dram_tensor("h_new", [DEV_NODES, HIDDEN], BF16, kind="ExternalOutput")

    with tile.TileContext(nc) as tc:
        with (
            tc.tile_pool(name="const", bufs=1) as cp,
        ):
            # ---- persistent constants ----
            iota32 = cp.tile([128, NQBmax, QW], BF16)
            # values 0..QW-1 repeated per quarter-block; exact in bf16
            nc.gpsimd.iota(iota32[:], pattern=[[0, NQBmax], [1, QW]], base=0,
                           channel_multiplier=0,
                           allow_small_or_imprecise_dtypes=True)
            dslot_t = cp.tile([128, NQB_TOT], BF16)
            nc.scalar.dma_start(out=dslot_t[:], in_=dstslot[:])
            wd_t = cp.tile([128, 2, MSG], FP8)
            nc.sync.dma_start(out=wd_t[:], in_=wd8T.rearrange("(a p) j -> p a j", p=128))
            waT_t = cp.tile([128, 384], BF16)
            nc.sync.dma_start(out=waT_t[:], in_=waT[:])
            wbT_t = cp.tile([128, 384], BF16)
            nc.sync.dma_start(out=wbT_t[:], in_=wbT[:])
            whhT_t = cp.tile([128, 384], BF16)
            nc.sync.dma_start(out=whhT_t[:], in_=whhT[:])
            src16_t = cp.tile([128, EDEV // 16], I16)
            # biggest constant (0.7MB): keep it off the sync queue so it
            # overlaps the first fh8 feature loads in a single-eval launch
            nc.scalar.dma_start(out=src16_t[:], in_=src16[:])
            if not zero_bias:
                bias_rz_t = cp.tile([128, 256], F32)
                nc.sync.dma_start(out=bias_rz_t[:], in_=bias_rz[:])
                bias_n_t = cp.tile([128, 128], F32)
                nc.sync.dma_start(out=bias_n_t[:], in_=bias_n[:])
                bias_hn_t = cp.tile([128, 128], F32)
                nc.sync.dma_start(out=bias_hn_t[:], in_=bias_hn[:])
            cT_tiles = []
            for w in range(WINS):
                ct = cp.tile([128, 128], BF16, tag=f"cT{w}")
                cT_tiles.append(ct)

            # Zd table: row r=(n%128)*ZBLKS + n//128 (partition-major), 256
            # fp8 cols (0:64 = zd, 64:256 junk pad to reach the 256B-row
            # minimum of dma_gather).
            ZROW = 256
            Zd = nc.dram_tensor("Zd", [ZPAD, ZROW], FP8)
            Zdv = Zd.rearrange("(p g) j -> p (g j)", p=128)  # [128, ZBLKS*256]

            # ---- Phase Z: zd = featH_fp8 @ Wd_fp8.T (all nodes) ----
            ZG = zg_group
            def emit_z_phase():
             with (
                tc.tile_pool(name="zio", bufs=4) as zio,
                tc.tile_pool(name="zps", bufs=3, space="PSUM") as zps,
             ):
              engs = [nc.sync, nc.scalar]
              for gi_, g0 in enumerate(range(0, ZBLKS, ZG)):
                gn = min(ZG, ZBLKS - g0)
                fg = zio.tile([128, ZG, 2, 128], FP8, tag="fg")
                cols = gn * 256
                engs[gi_ % 2].dma_start(
                    out=fg[:].rearrange("p g a n -> p (g a n)")[:, :cols],
                    in_=fh8[:, g0 * 256 : g0 * 256 + cols])
                zp = zps.tile([128, ZG * MSG], F32, space="PSUM", tag="zp")
                if not skip_zmm:
                    for zi in range(gn):
                        zslc = zp[:, zi * MSG : (zi + 1) * MSG]
                        nc.tensor.matmul(out=zslc, lhsT=fg[:, zi, 0, :],
                                         rhs=wd_t[:, 0, :], start=True, stop=False)
                        nc.tensor.matmul(out=zslc, lhsT=fg[:, zi, 1, :],
                                         rhs=wd_t[:, 1, :], start=False, stop=True)
                else:
                    nc.vector.memset(zp[:, : gn * MSG], 0.0)
                zs = zio.tile([128, ZG, MSG], FP8, tag="zs")
                nc.scalar.copy(
                    out=zs[:, :gn, :],
                    in_=zp[:, : gn * MSG].rearrange("p (g j) -> p g j", g=gn))
                if not skip_zwrite:
                    weng = engs[(gi_ + 1) % 2]
                    weng.dma_start(
                        out=Zd.rearrange("(p g) j -> p g j", p=128)[:, g0 : g0 + gn, 0:MSG],
                        in_=zs[:, :gn, :])

            # ---- Phase MSG + GRU, interleaved ----
            WG = gru_wg
            qctr = [0]
            mp = pp = gp = mps = rp = rps = rps2 = None
            def emit_msg_window(w):
                # P one-hot first: no gather dependency, so the in-order DVE
                # queue builds it while this window's gather is in flight
                nqb_w = NQB[w]
                woff = qoff_tab[w][0]
                P = pp.tile([128, NQBmax, QW], FP8, tag="P")
                nc.vector.tensor_tensor(
                    out=P[:, :nqb_w, :],
                    in0=iota32[:, :nqb_w, :],
                    in1=dslot_t[:, woff : woff + nqb_w, None]
                        .to_broadcast([128, nqb_w, QW]),
                    op=OP.is_equal)
                zg = gp.tile([128, B, ZROW], FP8, tag="zg")
                if skip_gather == "noinit":
                    pass
                elif skip_gather:
                    nc.vector.memset(zg[:], 0.0)
                else:
                    off = 0
                    while off < EW:
                        chunk = min(GCH, EW - off)
                        nc.gpsimd.dma_gather(
                            zg[:, off // 128 : (off + chunk) // 128, :], Zd[:],
                            src16_t[:, (w * EW + off) // 16 : (w * EW + off + chunk) // 16],
                            num_idxs=chunk, num_idxs_reg=chunk, elem_size=ZROW,
                            queue_num=qctr[0] % 4,
                        )
                        qctr[0] += 1
                        off += chunk
                dgw = gp.tile([128, B, MSG], FP8, tag="dgw")
                ueng = nc.sync if w % 2 == 0 else nc.scalar
                ueng.dma_start(
                    out=dgw[:].rearrange("p b c -> p (b c)"),
                    in_=dg_g[:, w * B * MSG : (w + 1) * B * MSG])
                m = mp.tile([128, B, TWO_MSG], FP8, tag="m")
                if chunked_compare:
                    off = 0
                    while off < EW:
                        chunk = min(GCH, EW - off)
                        b0, b1 = off // 128, (off + chunk) // 128
                        nc.vector.tensor_tensor(
                            out=m[:, b0:b1, 0:MSG], in0=zg[:, b0:b1, 0:MSG],
                            in1=dgw[:, b0:b1, :], op=OP.is_ge)
                        nc.vector.tensor_scalar(
                            out=m[:, b0:b1, MSG:TWO_MSG], in0=m[:, b0:b1, 0:MSG],
                            scalar1=1.0, scalar2=None, op0=OP.is_lt)
                        off += chunk
                else:
                    nc.vector.tensor_tensor(out=m[:, :, 0:MSG], in0=zg[:, :, 0:MSG],
                                            in1=dgw[:], op=OP.is_ge)
                    # m1 = 1 - m0 == (m0 < 1)
                    nc.vector.tensor_scalar(out=m[:, :, MSG:TWO_MSG],
                                            in0=m[:, :, 0:MSG],
                                            scalar1=1.0, scalar2=None, op0=OP.is_lt)
                cps = mps.tile([128, 128], F32, space="PSUM", tag="cps")
                for q in range(4):
                    qb0, qb1 = qranges[w][q]
                    creg = cps[:, QW * q : QW * (q + 1)]
                    if qb1 <= qb0 or skip_msgmm:
                        nc.vector.memset(creg, 0.0)
                        continue
                    po = qoff_tab[w][q] - woff
                    for b in range(qb0, qb1):
                        nc.tensor.matmul(out=creg, lhsT=m[:, b, :],
                                         rhs=P[:, po + (b - qb0), :],
                                         start=(b == qb0), stop=(b == qb1 - 1))
                # c = (c_sum > 0) == Sign(c_sum) since c_sum >= 0; runs on ACT
                nc.scalar.sign(out=cT_tiles[w][:], in_=cps[:])

            def emit_gru_group(w0):
                xh = rp.tile([128, WG, 2, 128], BF16, tag="xh")
                nc.sync.dma_start(
                    out=xh[:].rearrange("p w a n -> p (w a n)"),
                    in_=fh_locT[:, w0 * 256 : (w0 + WG) * 256])
                hl = rp.tile([128, WG, 128], BF16, tag="hl")
                for wi in range(WG):
                    w = w0 + wi
                    nc.scalar.dma_start(
                        out=hl[:WIN_NODES, wi, :],
                        in_=h_loc[w * WIN_NODES : (w + 1) * WIN_NODES, :])
                # two PSUM groups per window (rz gates / n gate) so each
                # group is opened and closed over exactly the same col range
                gi = rps.tile([128, WG, 256], F32, space="PSUM", tag="gi")
                gn_ps = rps.tile([128, WG, 128], F32, space="PSUM", tag="gn_ps")
                hn_ps = rps2.tile([128, WG, 128], F32, space="PSUM", tag="hn_ps")
                for wi in range(WG):
                    w = w0 + wi
                    nc.tensor.matmul(out=gi[:, wi, :], lhsT=xh[:, wi, 0, :],
                                     rhs=waT_t[:, 0:256], start=True, stop=False)
                    nc.tensor.matmul(out=gi[:, wi, :], lhsT=cT_tiles[w][:],
                                     rhs=wbT_t[:, 0:256], start=False, stop=False)
                    nc.tensor.matmul(out=gi[:, wi, :], lhsT=xh[:, wi, 1, :],
                                     rhs=whhT_t[:, 0:256], start=False, stop=True)
                    nc.tensor.matmul(out=gn_ps[:, wi, :], lhsT=xh[:, wi, 0, :],
                                     rhs=waT_t[:, 256:384], start=True, stop=False)
                    nc.tensor.matmul(out=gn_ps[:, wi, :], lhsT=cT_tiles[w][:],
                                     rhs=wbT_t[:, 256:384], start=False, stop=True)
                    nc.tensor.matmul(out=hn_ps[:, wi, :], lhsT=xh[:, wi, 1, :],
                                     rhs=whhT_t[:, 256:384], start=True, stop=True)
                V = WIN_NODES
                rz_s = rp.tile([128, WG, 256], F32, tag="rz_s")
                if zero_bias:
                    nc.scalar.activation(rz_s[:V], gi[:V], AF.Sigmoid)
                    rhn = rp.tile([128, WG, 128], F32, tag="rhn")
                    nc.vector.tensor_tensor(out=rhn[:V], in0=rz_s[:V, :, 0:128],
                                            in1=hn_ps[:V], op=OP.mult)
                    narg = rp.tile([128, WG, 128], F32, tag="narg")
                    nc.vector.tensor_tensor(out=narg[:V], in0=rhn[:V],
                                            in1=gn_ps[:V], op=OP.add)
                else:
                    rz = rp.tile([128, WG, 256], F32, tag="rz")
                    nc.vector.tensor_tensor(
                        out=rz[:V], in0=gi[:V],
                        in1=bias_rz_t[:V, None, :].to_broadcast([V, WG, 256]), op=OP.add)
                    nc.scalar.activation(rz_s[:V], rz[:V], AF.Sigmoid)
                    hn = rp.tile([128, WG, 128], F32, tag="hn")
                    nc.vector.tensor_tensor(
                        out=hn[:V], in0=hn_ps[:V],
                        in1=bias_hn_t[:V, None, :].to_broadcast([V, WG, 128]), op=OP.add)
                    inn = rp.tile([128, WG, 128], F32, tag="inn")
                    nc.vector.tensor_tensor(
                        out=inn[:V], in0=gn_ps[:V],
                        in1=bias_n_t[:V, None, :].to_broadcast([V, WG, 128]), op=OP.add)
                    rhn = rp.tile([128, WG, 128], F32, tag="rhn")
                    nc.vector.tensor_tensor(out=rhn[:V], in0=rz_s[:V, :, 0:128], in1=hn[:V], op=OP.mult)
                    narg = rp.tile([128, WG, 128], F32, tag="narg")
                    nc.vector.tensor_tensor(out=narg[:V], in0=inn[:V], in1=rhn[:V], op=OP.add)
                n_t = rp.tile([128, WG, 128], F32, tag="n_t")
                nc.scalar.activation(n_t[:V], narg[:V], AF.Tanh)
                hmn = rp.tile([128, WG, 128], F32, tag="hmn")
                nc.vector.tensor_tensor(out=hmn[:V], in0=hl[:V], in1=n_t[:V], op=OP.subtract)
                zh = rp.tile([128, WG, 128], F32, tag="zh")
                nc.vector.tensor_tensor(out=zh[:V], in0=rz_s[:V, :, 128:256], in1=hmn[:V], op=OP.mult)
                ho = rp.tile([128, WG, 128], BF16, tag="ho")
                nc.vector.tensor_tensor(out=ho[:V], in0=n_t[:V], in1=zh[:V], op=OP.add)
                for wi in range(WG):
                    w = w0 + wi
                    nc.sync.dma_start(
                        out=h_new[w * WIN_NODES : (w + 1) * WIN_NODES, :],
                        in_=ho[:WIN_NODES, wi, :])

            def emit_eval():
                nonlocal mp, pp, gp, mps, rp, rps, rps2
                if "z" in phases:
                    emit_z_phase()
                with (
                    tc.tile_pool(name="msg", bufs=2) as mp,
                    tc.tile_pool(name="ponehot", bufs=3) as pp,
                    tc.tile_pool(name="gat", bufs=3) as gp,
                    tc.tile_pool(name="mps", bufs=2, space="PSUM") as mps,
                    tc.tile_pool(name="gru", bufs=2) as rp,
                    tc.tile_pool(name="rps", bufs=1, space="PSUM") as rps,
                    tc.tile_pool(name="rps2", bufs=2, space="PSUM") as rps2,
                ):
                    for w in range(WINS):
                        if "m" in phases:
                            emit_msg_window(w)
                        if "g" in phases and w % WG == WG - 1:
                            emit_gru_group(w - WG + 1)

            if repeats > 1 and hw_loop:
                U = loop_unroll
                assert repeats % U == 0
                with tc.For_i(0, repeats // U):
                    for _u in range(U):
                        emit_eval()
            else:
                for _rep in range(repeats):
                    emit_eval()

    nc.compile()
    return nc


# message-column permutation: evens first, then odds
PERM = np.concatenate([np.arange(0, TWO_MSG, 2), np.arange(1, TWO_MSG, 2)])


def _prep_host(feat, h, src, dst, u, dg_bias=None):
    """Host-side sharding/layout. Returns (B, list of per-core in_maps)."""
    import ml_dtypes
    bf16 = ml_dtypes.bfloat16
    fp8 = ml_dtypes.float8_e4m3

    feat = np.ascontiguousarray(feat, dtype=np.float32)
    h = np.ascontiguousarray(h, dtype=np.float32)
    src = np.asarray(src).astype(np.int64)
    dst = np.asarray(dst).astype(np.int64)
    u = np.asarray(u, dtype=np.float32)

    featH = np.concatenate([feat, h], axis=1)  # [N, 256]
    featH_pad = np.zeros((ZPAD, 256), np.float32)
    featH_pad[:N_NODES] = featH
    # channel-major fp8: [p, zb, a, n] = featH_pad[zb*128+n, a*128+p]
    fh8 = np.ascontiguousarray(
        featH_pad.astype(fp8).reshape(ZBLKS, 128, 2, 128).transpose(3, 0, 2, 1)
    ).reshape(128, -1)

    # host Gumbel difference, folding b_enc (b_enc=0 here but stay general at
    # the call site via _prep_weights -> dg_bias)
    dgf = (np.log(-np.log(u[..., 0].astype(np.float64) + EPS) + EPS)
           - np.log(-np.log(u[..., 1].astype(np.float64) + EPS) + EPS))
    if dg_bias is not None:
        dgf = dgf - np.asarray(dg_bias, np.float64)[None, :]

    order = np.argsort(dst, kind="stable")
    dst_s = dst[order]
    src_s = src[order]
    win = dst_s // WIN_NODES                     # window id per sorted edge
    counts = np.bincount(win, minlength=N_WINDOWS)
    starts = np.zeros(N_WINDOWS + 1, np.int64)
    np.cumsum(counts, out=starts[1:])
    B = int(np.max((counts + 127) // 128))
    B = max(B, 1)
    EW = B * 128
    EDEV = WINS * EW

    # per (window-in-device, quarter): block range [qb0, qb1) containing any
    # edge with dstslot in [QW*q, QW*(q+1)); SPMD-uniform = min/max over devs
    qb0 = np.full((NDEV, WINS, 4), B, np.int64)
    qb1 = np.zeros((NDEV, WINS, 4), np.int64)
    for wid in range(N_WINDOWS):
        d, k = divmod(wid, WINS)
        s, e = starts[wid], starts[wid + 1]
        slots = dst_s[s:e] - wid * WIN_NODES     # ascending within window
        qidx = slots // QW
        pos = np.arange(e - s)
        for q in range(4):
            sel = pos[qidx == q]
            if len(sel):
                qb0[d, k, q] = sel[0] // 128
                qb1[d, k, q] = sel[-1] // 128 + 1
    qb0g = qb0.min(axis=0)
    qb1g = qb1.max(axis=0)
    empty = qb1g <= qb0g
    qb0g[empty] = 0
    qb1g[empty] = 0
    qranges = tuple(tuple((int(qb0g[k, q]), int(qb1g[k, q])) for q in range(4))
                    for k in range(WINS))

    in_maps = []
    for d in range(NDEV):
        src_pad = np.zeros((EDEV,), np.int64)
        slot_pad = np.full((EDEV,), -1.0, np.float32)
        dg_pad = np.zeros((EDEV, MSG), np.float32)
        for k in range(WINS):
            wid = d * WINS + k
            s, e = starts[wid], starts[wid + 1]
            n = e - s
            o = k * EW
            src_pad[o : o + n] = src_s[s:e]
            slot_pad[o : o + n] = (dst_s[s:e] - wid * WIN_NODES).astype(np.float32)
            dg_pad[o : o + n] = dgf[order[s:e]]

        # partition-major table index: row of node n is (n%128)*ZBLKS + n//128
        srcT = (src_pad % 128) * ZBLKS + src_pad // 128
        # gather idx layout: [p, s] = idx[16*s + p%16], replicated across groups
        idx16 = np.empty((128, EDEV // 16), np.int16)
        flat = srcT.astype(np.int16).reshape(EDEV // 16, 16).T  # [16, EDEV/16]
        for g in range(8):
            idx16[g * 16 : (g + 1) * 16, :] = flat
        # quarter-shifted compact dstslot: one 128-row per (w, q, b-in-range)
        # col; value = slot - QW*q (values outside [0, QW) match no one-hot)
        slot_w = slot_pad.reshape(WINS, B, 128)
        cols = []
        for k in range(WINS):
            for q in range(4):
                b0, b1 = qranges[k][q]
                if b1 > b0:
                    cols.append(slot_w[k, b0:b1, :] - QW * q)
        dstslot_c = np.ascontiguousarray(
            np.concatenate(cols, axis=0).T.astype(bf16))  # [128, NQB_TOT]
        # dg swizzled: [p, blk*64 + c] = dg_pad[blk*128 + p, c]
        dg_sw = np.ascontiguousarray(
            dg_pad.astype(fp8).reshape(EDEV // 128, 128, MSG)
            .transpose(1, 0, 2)).reshape(128, -1)
        # local featH channel-major bf16: [p, w, a, n] = featH[node, a*128+p]
        base = d * DEV_NODES
        end = min(base + DEV_NODES, N_NODES)
        locflat = np.zeros((DEV_NODES, 256), np.float32)
        locflat[: end - base] = featH[base:end]
        loc = np.zeros((WINS, 128, 2, 128), np.float32)  # [w, n, a, p]
        loc[:, :WIN_NODES] = locflat.reshape(WINS, WIN_NODES, 2, 128)
        fh_locT = np.ascontiguousarray(
            loc.astype(bf16).transpose(3, 0, 2, 1)).reshape(128, -1)
        h_pad = np.zeros((DEV_NODES, HIDDEN), np.float32)
        h_pad[: end - base] = h[base:end]
        h_loc = np.ascontiguousarray(h_pad.astype(bf16))
        in_maps.append({
            "fh8": fh8, "fh_locT": fh_locT, "h_loc": h_loc,
            "dg_g": dg_sw, "src16": idx16, "dstslot": dstslot_c,
        })
    return B, qranges, in_maps


def _prep_weights(W_enc, b_enc, W_dec, b_dec, W_ih, W_hh, b_ih, b_hh):
    import ml_dtypes
    bf16 = ml_dtypes.bfloat16
    fp8 = ml_dtypes.float8_e4m3

    W_enc = np.asarray(W_enc, np.float32)
    W_dec = np.asarray(W_dec, np.float32)
    W_ih = np.asarray(W_ih, np.float32)
    W_hh = np.asarray(W_hh, np.float32)
    b_enc = np.asarray(b_enc, np.float32)
    b_dec = np.asarray(b_dec, np.float32)
    b_ih = np.asarray(b_ih, np.float32)
    b_hh = np.asarray(b_hh, np.float32)

    Wd = W_enc[0::2].astype(np.float64) - W_enc[1::2].astype(np.float64)  # [64, 256]
    wd8T = np.ascontiguousarray(Wd.T.astype(fp8))            # [256, 64]

    W_b = (W_ih[:, HIDDEN:].astype(np.float64) @ W_dec.astype(np.float64))
    b_comb = (W_ih[:, HIDDEN:].astype(np.float64) @ b_dec.astype(np.float64)) + b_ih

    waT = np.ascontiguousarray(W_ih[:, :HIDDEN].T.astype(bf16))      # [128, 384]
    wbT = np.ascontiguousarray(W_b.T.astype(np.float32)[PERM, :].astype(bf16))
    whhT = np.ascontiguousarray(W_hh.T.astype(bf16))                 # [128, 384]
    brz = (b_comb[:256] + b_hh[:256]).astype(np.float32)
    bn = b_comb[256:384].astype(np.float32)
    bhn = b_hh[256:384].astype(np.float32)
    return {
        "wd8T": wd8T,
        "waT": waT, "wbT": wbT, "whhT": whhT,
        "bias_rz": np.ascontiguousarray(np.tile(brz, (128, 1))),
        "bias_n": np.ascontiguousarray(np.tile(bn, (128, 1))),
        "bias_hn": np.ascontiguousarray(np.tile(bhn, (128, 1))),
    }, (b_enc[0::2].astype(np.float64) - b_enc[1::2].astype(np.float64))


def kernel(feat, h, src, dst, u, W_enc, b_enc, W_dec, b_dec, W_ih, W_hh,
           b_ih, b_hh):
    wmap, dg_bias = _prep_weights(W_enc, b_enc, W_dec, b_dec, W_ih, W_hh,
                                  b_ih, b_hh)
    B, qranges, in_maps = _prep_host(feat, h, src, dst, u, dg_bias=dg_bias)
    for m in in_maps:
        m.update(wmap)

    phases = os.environ.get("KERNEL_PHASES", "zmg")
    zero_bias = not (np.any(np.asarray(b_dec)) or np.any(np.asarray(b_ih))
                     or np.any(np.asarray(b_hh)))
    key = (B, qranges, phases, zero_bias)
    if key not in _cache:
        _cache[key] = build_program(B, qranges, phases, zero_bias)
    nc = _cache[key]

    res = run_bass_kernel_spmd(nc, in_maps, core_ids=list(range(NDEV)))
    h_new = np.concatenate([res.results[d]["h_new"] for d in range(NDEV)],
                           axis=0)[:N_NODES].astype(np.float32)
    return (h_new, h_new)

